# revision 32
# baseline (speedup 1.0000x reference)
"""Trainium2 Bass kernel for nn_AdaLNSelfAttn_RoPE (B=4, L=2048, C=1024, H=16).

Sharding across 8 NeuronCores (one chip):
  - Tokens flattened (B*L = 8192) and sharded 1024/core; each core owns half of
    one batch.  LayerNorms, AdaLN modulation, proj, and the FFN run on the
    token shard (sequence parallel).
  - QKV + attention are head-sharded: core m owns heads {2m, 2m+1}.  The
    modulated hidden h1 is AllGather'ed (bf16) so every core computes QKV for
    its two heads over all tokens.  Attention outputs return to token shards
    via AllToAll.
  - AdaLN (silu+linear) is column-sharded across pairs of cores (both cores of
    a pair own the same batch) with a 2-wide AllGather.
  - Activations are channel-major (C on partitions) so matmuls chain without
    transposes; per-token statistics (LN mean/rstd, softmax rowsums, q-norms)
    are reduced across partitions with ones-matmuls and broadcast back with
    outer-product matmuls.
  - Matmuls in bf16 (fp32 PSUM accumulation); statistics in fp32.
  - Softmax skips max-subtraction (cosine attention bounds scores to [-4,4]).
    The softmax denominator rides as a 65th output column of the PV matmul;
    the k-side 1/||k|| rides as the per-partition scale of the exp()
    activation.
  - RoPE pairs are pre-permuted on the host (re parts in rows 0-31 of each
    head slice, im parts in rows 32-63) so rotation is four 32-row block ops.
"""

import numpy as np

import concourse.bass as bass
import concourse.bacc as bacc
import concourse.mybir as mybir
import concourse.tile as tile
from concourse.bass_utils import run_bass_kernel_spmd

F32 = mybir.dt.float32
BF16 = mybir.dt.bfloat16
AF = mybir.ActivationFunctionType
ALU = mybir.AluOpType

B, L, C, H = 4, 2048, 1024, 16
HD = C // H          # 64
N_CORES = 8
TOK = (B * L) // N_CORES   # 1024 tokens per core
MAX_SCALE = float(np.log(100.0))
LN_EPS = 1e-6
P = 128


def build_nc(debug_outputs=()):
    nc = bacc.Bacc("TRN2", target_bir_lowering=False, debug=False,
                   num_devices=N_CORES)

    dt = nc.dram_tensor
    io = {}
    io["xT"] = dt("xT", [C, TOK], F32, kind="ExternalInput")
    io["condT"] = dt("condT", [C, 1], F32, kind="ExternalInput")
    io["w_ada"] = dt("w_ada", [C, 3 * C], BF16, kind="ExternalInput")
    io["b_ada_r"] = dt("b_ada_r", [1, 3 * C], F32, kind="ExternalInput")
    io["w_qkv"] = dt("w_qkv", [C, 384], BF16, kind="ExternalInput")
    io["qkv_b"] = dt("qkv_b", [384, 1], F32, kind="ExternalInput")
    io["perm_m"] = dt("perm_m", [P, P], BF16, kind="ExternalInput")
    io["sel2t"] = dt("sel2t", [2, P], BF16, kind="ExternalInput")
    io["ident"] = dt("ident", [P, P], BF16, kind="ExternalInput")
    io["vb2"] = dt("vb2", [64, 2], F32, kind="ExternalInput")
    io["vb128"] = dt("vb128", [P, 1], F32, kind="ExternalInput")
    io["scale_log"] = dt("scale_log", [2, 1], F32, kind="ExternalInput")
    io["cosT"] = dt("cosT", [32, L], F32, kind="ExternalInput")
    io["sinT"] = dt("sinT", [32, L], F32, kind="ExternalInput")
    io["w_proj"] = dt("w_proj", [C, C], BF16, kind="ExternalInput")
    io["b_proj_c"] = dt("b_proj_c", [P, 8], F32, kind="ExternalInput")
    io["w_fc1"] = dt("w_fc1", [C, 4 * C], BF16, kind="ExternalInput")
    io["b_fc1_c"] = dt("b_fc1_c", [P, 32], F32, kind="ExternalInput")
    io["w_fc2"] = dt("w_fc2", [4 * C, C], BF16, kind="ExternalInput")
    io["b_fc2_c"] = dt("b_fc2_c", [P, 8], F32, kind="ExternalInput")
    io["outT"] = dt("outT", [C, TOK], F32, kind="ExternalOutput")

    dbg = {}
    for name, shape, dtp in [
        ("h1_all", [8, C, TOK], BF16),
        ("qn", [P, B * L], BF16),
        ("kn", [P, B * L], BF16),
        ("v_sb", [P, B * 16 * 2 * 65], BF16),
        ("attn", [P, B * L], BF16),
        ("a2a_out", [8, P, TOK], BF16),
        ("x2", [P, 8 * TOK], F32),
        ("ada_all", [2, 3 * C], F32),
    ]:
        if name in debug_outputs:
            dbg[name] = dt("dbg_" + name, shape, dtp, kind="ExternalOutput")

    with tile.TileContext(nc) as tc:
        _body(nc, tc, io, dbg)
    nc.compile()
    return nc


def _body(nc, tc, io, dbg):
    mm = nc.tensor.matmul
    V = nc.vector
    S = nc.scalar
    dma = nc.sync.dma_start

    def mm1(out, lhsT, rhs):
        mm(out, lhsT, rhs, start=True, stop=True)

    with tc.tile_pool(name="dram", bufs=1, space="DRAM") as dram, \
         tc.tile_pool(name="const", bufs=1) as const, \
         tc.tile_pool(name="outer", bufs=1) as outer:

        # DRAM bounce buffers for collectives
        ada_in = dram.tile([1, 3 * C], F32, name="ada_in")
        ada_all = dram.tile([2, 3 * C], F32, name="ada_all")
        h1_in = dram.tile([TOK, C], BF16, name="h1_in")
        h1_all = [dram.tile([8, 2 * P, TOK], BF16, addr_space="Shared",
                            name=f"h1_all{c}") for c in range(4)]
        a2a_in = dram.tile([8, P, TOK], BF16, name="a2a_in")
        a2a_out = dram.tile([8, P, TOK], BF16, name="a2a_out")

        # ------------------------------------------------------- constants
        ones128 = const.tile([P, 1], BF16, name="ones128")
        V.memset(ones128[:], 1.0)
        ones64c = const.tile([64, 1], BF16, name="ones64c")
        V.memset(ones64c[:], 1.0)
        ones1x64 = const.tile([1, 64], BF16, name="ones1x64")
        V.memset(ones1x64[:], 1.0)
        sel2 = const.tile([P, 2], BF16, name="sel2")     # head indicator cols
        V.memset(sel2[:], 0.0)
        V.memset(sel2[0:64, 0:1], 1.0)
        V.memset(sel2[64:128, 1:2], 1.0)
        sel2T = const.tile([2, P], BF16, name="sel2T")   # head indicator rows
        dma(out=sel2T[:], in_=io["sel2t"][:, :])

        # rope cos (128, 2048) bf16: 4 vertical copies of (32, 2048); and
        # sign-baked sin: rows [0:32]=-sin [32:64]=+sin [64:96]=-sin [96:]=+sin
        Cb = const.tile([P, L], BF16, name="Cb")
        Sb = const.tile([P, L], BF16, name="Sb")
        with tc.tile_pool(name="cs_pool", bufs=1) as csp:
            cs_f = csp.tile([P, L], F32, name="cs_f")
            for j in range(4):
                dma(out=cs_f[32 * j:32 * (j + 1), :], in_=io["cosT"][:, :])
            V.tensor_copy(Cb[:], cs_f[:])
            sn_f = csp.tile([P, L], F32, name="sn_f")
            for j in range(4):
                dma(out=sn_f[32 * j:32 * (j + 1), :], in_=io["sinT"][:, :])
            for j in range(4):
                sgn = -1.0 if j % 2 == 0 else 1.0
                S.activation(Sb[32 * j:32 * (j + 1), :],
                             sn_f[32 * j:32 * (j + 1), :], AF.Copy, scale=sgn)
        perm_sb = const.tile([P, P], BF16, name="perm_sb")
        dma(out=perm_sb[:], in_=io["perm_m"][:, :])
        ident_sb = const.tile([P, P], BF16, name="ident_sb")
        dma(out=ident_sb[:], in_=io["ident"][:, :])
        vb2 = const.tile([64, 2], F32, name="vb2")
        dma(out=vb2[:], in_=io["vb2"][:, :])
        vb128 = const.tile([P, 1], F32, name="vb128")
        dma(out=vb128[:], in_=io["vb128"][:, :])
        ones65 = const.tile([65, 64], BF16, name="ones65")
        V.memset(ones65[:], 1.0)

        # bias columns
        qb_col = const.tile([P, 3], F32, name="qb_col")
        dma(out=qb_col[:], in_=io["qkv_b"].ap().rearrange("(m p) 1 -> p m", p=P))
        bproj = const.tile([P, 8], F32, name="bproj")
        dma(out=bproj[:], in_=io["b_proj_c"][:, :])
        bfc1 = const.tile([P, 32], F32, name="bfc1")
        dma(out=bfc1[:], in_=io["b_fc1_c"][:, :])
        bfc2 = const.tile([P, 8], F32, name="bfc2")
        dma(out=bfc2[:], in_=io["b_fc2_c"][:, :])
        bada = const.tile([1, 3 * C], F32, name="bada")
        dma(out=bada[:], in_=io["b_ada_r"][:, :])
        epsc = const.tile([1, 1], F32, name="epsc")
        V.memset(epsc[:], LN_EPS)

        # scale_mul = exp(min(scale_log, MAX_SCALE))
        scale_sb = const.tile([2, 1], F32, name="scale_sb")
        with tc.tile_pool(name="scp", bufs=1) as scp:
            sc_raw = scp.tile([2, 1], F32, name="sc_raw")
            dma(out=sc_raw[:], in_=io["scale_log"][:, :])
            sc_min = scp.tile([2, 1], F32, name="sc_min")
            V.tensor_scalar_min(sc_min[:], sc_raw[:], MAX_SCALE)
            S.activation(scale_sb[:], sc_min[:], AF.Exp)

        # --------------------------------------------------- phase 0: adaLN
        ada_phase(nc, tc, io, dram, const, ada_in, ada_all)

        g1c = ada_col(nc, const, ada_all, 0, "g1c")
        g2c = ada_col(nc, const, ada_all, 1, "g2c")
        sh1c = ada_col(nc, const, ada_all, 4, "sh1c")
        sh2c = ada_col(nc, const, ada_all, 5, "sh2c")
        s1p = ada_srow(nc, tc, const, ada_all, 2, "s1p")
        s2p = ada_srow(nc, tc, const, ada_all, 3, "s2p")

        if "ada_all" in dbg:
            dma(out=dbg["ada_all"][:], in_=ada_all[:])

        # ------------------------------------------------------ LN1 -> AG h1
        with tc.tile_pool(name="ln1pool", bufs=1) as lp, \
             tc.tile_pool(name="ln1psum", bufs=2, space="PSUM") as lps, \
             tc.tile_pool(name="ln1tmp", bufs=3) as ltmp:
            xT_sb = lp.tile([P, 8 * TOK], F32, name="xT_sb")
            dma(out=xT_sb[:].rearrange("p (blk t) -> p blk t", blk=8),
                in_=io["xT"].ap().rearrange("(blk p) t -> p blk t", p=P))
            h1_bf = lp.tile([P, 8 * TOK], BF16, name="h1_bf")

            def h1_chunk_out(ci2):
                # after channel blocks 2*ci2, 2*ci2+1 are ready: stage + AG
                csl = slice(2 * ci2 * P, 2 * (ci2 + 1) * P)
                dma(out=h1_in[csl, :].rearrange("(blk p) t -> p blk t", p=P),
                    in_=h1_bf[:, 2 * ci2 * TOK:2 * (ci2 + 1) * TOK]
                    .rearrange("p (blk t) -> p blk t", blk=2))
                nc.gpsimd.collective_compute(
                    "AllGather", ALU.bypass,
                    replica_groups=[list(range(N_CORES))],
                    ins=[h1_in[csl, :].opt()],
                    outs=[h1_all[ci2][:].opt()])

            layer_norm(nc, lp, lps, ltmp, xT_sb, s1p, sh1c, h1_bf,
                       ones128, epsc, mm, block_done=h1_chunk_out)
        if "h1_all" in dbg:
            for c in range(4):
                dma(out=dbg["h1_all"][:, 2 * c * P:2 * (c + 1) * P, :],
                    in_=h1_all[c][:])

        # ------------------------------------- phase 2+3: qkv, rope, attention
        with tc.tile_pool(name="atslab", bufs=1) as ats, \
             tc.tile_pool(name="attmp", bufs=2) as atmp, \
             tc.tile_pool(name="atrhs", bufs=18) as arhs:
            qkv_phase(nc, tc, io, dbg, ats, atmp, arhs,
                      h1_all, a2a_in, mm, mm1,
                      Cb, Sb, sel2, sel2T, ones128, ones65,
                      qb_col, vb128, perm_sb, ident_sb, scale_sb)
        nc.gpsimd.collective_compute(
            "AllToAll", ALU.bypass,
            replica_groups=[list(range(N_CORES))],
            ins=[a2a_in[:].opt()], outs=[a2a_out[:].opt()])
        if "a2a_out" in dbg:
            dma(out=dbg["a2a_out"][:], in_=a2a_out[:])

        # ----------------------------------------- phase 4: proj + residual
        x2p_cm = tc.tile_pool(name="x2pool", bufs=1)
        x2p = x2p_cm.__enter__()
        x2_sb = x2p.tile([P, 8 * TOK], F32, name="x2_sb")
        with tc.tile_pool(name="pjpool", bufs=1) as pjp, \
             tc.tile_pool(name="pjpsum", bufs=2, space="PSUM") as pjps, \
             tc.tile_pool(name="pjtmp", bufs=3) as ptmp:
            wp_sb = pjp.tile([P, 8 * C], BF16, name="wp_sb")
            dma(out=wp_sb[:].rearrange("p (blk c) -> p blk c", blk=8),
                in_=io["w_proj"].ap().rearrange("(blk p) c -> p blk c", p=P))
            prhs = []
            for r in range(8):
                for nt in range(2):
                    t = pjp.tile([P, 512], BF16, name=f"prhs_{r}_{nt}",
                                 tag="prhs", bufs=16)
                    dma(out=t[:], in_=a2a_out[r, :, nt * 512:(nt + 1) * 512])
                    prhs.append(t)
            for cot in range(8):
                xres = ptmp.tile([P, 2 * 512], F32, name="xres")
                dma(out=xres[:],
                    in_=io["xT"][cot * P:(cot + 1) * P, :])
                pp = [pjps.tile([P, 512], F32, name=f"p_ps{nt}",
                                tag=f"p_ps{nt}") for nt in range(2)]
                for r in range(8):
                    wt = wp_sb[:, r * C + cot * P:r * C + (cot + 1) * P]
                    for nt in range(2):
                        mm(pp[nt][:], wt, prhs[r * 2 + nt][:],
                           start=(r == 0), stop=(r == 7))
                for nt in range(2):
                    t1 = ptmp.tile([P, 512], F32, name="pj_t1")
                    V.tensor_scalar(t1[:], pp[nt][:], bproj[:, cot:cot + 1],
                                    g1c[:, cot:cot + 1], ALU.add, ALU.mult)
                    sl = slice(cot * TOK + nt * 512, cot * TOK + (nt + 1) * 512)
                    V.tensor_add(x2_sb[:, sl], t1[:],
                                 xres[:, nt * 512:(nt + 1) * 512])
        if "x2" in dbg:
            dma(out=dbg["x2"][:], in_=x2_sb[:])

        # ------------------------------------------------- phase 5-7: LN2+FFN
        with tc.tile_pool(name="ffnpool", bufs=1) as fp, \
             tc.tile_pool(name="ffntmp", bufs=3) as ftmp:
            h2_bf = fp.tile([P, 8 * TOK], BF16, name="h2_bf")
            with tc.tile_pool(name="ln2pool", bufs=1) as lp2, \
                 tc.tile_pool(name="ln2psum", bufs=2, space="PSUM") as lps2:
                layer_norm(nc, lp2, lps2, ftmp, x2_sb, s2p, sh2c, h2_bf,
                           ones128, epsc, mm)

            fw_cm = tc.tile_pool(name="ffnw", bufs=2)
            fw = fw_cm.__enter__()
            fps_cm = tc.tile_pool(name="ffnpsum", bufs=4, space="PSUM")
            fps = fps_cm.__enter__()
            hact = fp.tile([P, 32 * TOK], BF16, name="hact")
            for cot in range(32):
                w1 = fw.tile([P, 8 * P], BF16, name="w1")
                dma(out=w1[:].rearrange("p (blk c) -> p blk c", blk=8),
                    in_=io["w_fc1"].ap()[:, cot * P:(cot + 1) * P]
                    .rearrange("(blk p) c -> p blk c", p=P))
                fpp = [fps.tile([P, 512], F32, name=f"f_ps{nt}",
                                tag=f"f_ps{nt}", bufs=2) for nt in range(2)]
                for ci in range(8):
                    wt = w1[:, ci * P:(ci + 1) * P]
                    for nt in range(2):
                        mm(fpp[nt][:], wt,
                           h2_bf[:, ci * TOK + nt * 512:
                                 ci * TOK + (nt + 1) * 512],
                           start=(ci == 0), stop=(ci == 7))
                for nt in range(2):
                    S.activation(
                        hact[:, cot * TOK + nt * 512:cot * TOK + (nt + 1) * 512],
                        fpp[nt][:], AF.Gelu_apprx_tanh,
                        bias=bfc1[:, cot:cot + 1])

            for cot in range(8):
                w2 = fw.tile([P, 32 * P], BF16, name="w2")
                dma(out=w2[:].rearrange("p (blk c) -> p blk c", blk=32),
                    in_=io["w_fc2"].ap()[:, cot * P:(cot + 1) * P]
                    .rearrange("(blk p) c -> p blk c", p=P))
                opp = [fps.tile([P, 512], F32, name=f"o_ps{nt}",
                                tag=f"o_ps{nt}", bufs=2) for nt in range(2)]
                for ci in range(32):
                    wt = w2[:, ci * P:(ci + 1) * P]
                    for nt in range(2):
                        mm(opp[nt][:], wt,
                           hact[:, ci * TOK + nt * 512:
                                 ci * TOK + (nt + 1) * 512],
                           start=(ci == 0), stop=(ci == 31))
                for nt in range(2):
                    t1 = ftmp.tile([P, 512], F32, name="o_t1")
                    V.tensor_scalar(t1[:], opp[nt][:], bfc2[:, cot:cot + 1],
                                    g2c[:, cot:cot + 1], ALU.add, ALU.mult)
                    ot = ftmp.tile([P, 512], F32, name="ot")
                    sl = slice(cot * TOK + nt * 512, cot * TOK + (nt + 1) * 512)
                    V.tensor_add(ot[:], t1[:], x2_sb[:, sl])
                    dma(out=io["outT"][cot * P:(cot + 1) * P,
                                       nt * 512:(nt + 1) * 512],
                        in_=ot[:])
            fps_cm.__exit__(None, None, None)
            fw_cm.__exit__(None, None, None)
        x2p_cm.__exit__(None, None, None)


def ada_phase(nc, tc, io, dram, const, ada_in, ada_all):
    """silu(cond) @ W_ada_slice.T + b_ada, pair-wise AllGather."""
    mm = nc.tensor.matmul
    V = nc.vector
    S = nc.scalar
    dma = nc.sync.dma_start
    cond_sb = const.tile([P, 8], F32, name="cond_sb")
    dma(out=cond_sb[:],
        in_=io["condT"].ap().rearrange("(blk p) 1 -> p blk", p=P))
    scond = const.tile([P, 8], BF16, name="scond")
    S.activation(scond[:], cond_sb[:], AF.Silu)
    bada = const.tile([1, 3 * C], F32, name="bada2")
    dma(out=bada[:], in_=io["b_ada_r"][:, :])

    with tc.tile_pool(name="adapool", bufs=1) as ap_, \
         tc.tile_pool(name="adapsum", bufs=2, space="PSUM") as aps, \
         tc.tile_pool(name="adarhs", bufs=4) as arp:
        ada_sb = ap_.tile([1, 3 * C], F32, name="ada_sb")
        for nt6 in range(6):
            a_ps = aps.tile([1, 512], F32, name="a_ps")
            for ci in range(8):
                wt = arp.tile([P, 512], BF16, name="wt_ada")
                dma(out=wt[:], in_=io["w_ada"][ci * P:(ci + 1) * P,
                                               nt6 * 512:(nt6 + 1) * 512])
                mm(a_ps[:], scond[:, ci:ci + 1], wt[:],
                   start=(ci == 0), stop=(ci == 7))
            V.tensor_add(ada_sb[0:1, nt6 * 512:(nt6 + 1) * 512], a_ps[:],
                         bada[0:1, nt6 * 512:(nt6 + 1) * 512])
        dma(out=ada_in[:], in_=ada_sb[:])
    nc.gpsimd.collective_compute(
        "AllGather", ALU.bypass,
        replica_groups=[[0, 1], [2, 3], [4, 5], [6, 7]],
        ins=[ada_in[:].opt()], outs=[ada_all[:].opt()])


def ada_col(nc, const, ada_all, vec, name):
    r, off = (vec * C) // (3 * C), (vec * C) % (3 * C)
    t = const.tile([P, 8], F32, name=name)
    nc.sync.dma_start(out=t[:], in_=ada_all[r:r + 1, off:off + C]
                      .rearrange("1 (blk p) -> p blk", p=P))
    return t


def ada_srow(nc, tc, const, ada_all, vec, name):
    """(1, 1024) bf16 row of (ada_vec + 1)."""
    r, off = (vec * C) // (3 * C), (vec * C) % (3 * C)
    t = const.tile([1, C], BF16, name=name)
    with tc.tile_pool(name=name + "_f", bufs=1) as p:
        raw = p.tile([1, C], F32, name=name + "_raw")
        nc.sync.dma_start(out=raw[:], in_=ada_all[r:r + 1, off:off + C])
        nc.vector.tensor_scalar_add(t[:], raw[:], 1.0)
    return t


def layer_norm(nc, pool, psum, tmp, x_sb, sp_row, sh_col, out_bf, ones128, epsc, mm,
               block_done=None):
    """x_sb (128, 8192) f32, channel-major blocks; out_bf same layout bf16:
    LN(x) * (s+1) + sh, statistics over the channel (partition x block) dim."""
    V = nc.vector
    S = nc.scalar
    xc = pool.tile([P, 8 * TOK], BF16, name="ln_xc")
    for ci in range(8):
        sl = slice(ci * TOK, (ci + 1) * TOK)
        V.tensor_copy(xc[:, sl], x_sb[:, sl])
    mu_ps = [psum.tile([1, 512], F32, name=f"mu_ps{nt}", tag=f"mu_ps{nt}",
                       bufs=1) for nt in range(2)]
    s2_ps = [psum.tile([1, 512], F32, name=f"s2_ps{nt}", tag=f"s2_ps{nt}",
                       bufs=1) for nt in range(2)]
    for ci in range(8):
        sl = slice(ci * TOK, (ci + 1) * TOK)
        xsq = tmp.tile([P, TOK], BF16, name="ln_xsq")
        V.tensor_mul(xsq[:], xc[:, sl], xc[:, sl])
        for nt in range(2):
            tsl = slice(ci * TOK + nt * 512, ci * TOK + (nt + 1) * 512)
            mm(mu_ps[nt][:], ones128[:], xc[:, tsl],
               start=(ci == 0), stop=(ci == 7))
            mm(s2_ps[nt][:], ones128[:], xsq[:, nt * 512:(nt + 1) * 512],
               start=(ci == 0), stop=(ci == 7))
    mu = pool.tile([1, TOK], F32, name="ln_mu")
    va = pool.tile([1, TOK], F32, name="ln_va")
    for nt in range(2):
        tsl = slice(nt * 512, (nt + 1) * 512)
        S.activation(mu[0:1, tsl], mu_ps[nt][:], AF.Copy, scale=1.0 / C)
        S.activation(va[0:1, tsl], s2_ps[nt][:], AF.Copy, scale=1.0 / C)
    # va := rstd = 1/sqrt(va - mu^2 + eps), in place
    mu2 = pool.tile([1, TOK], F32, name="ln_mu2")
    V.tensor_mul(mu2[:], mu[:], mu[:])
    V.tensor_sub(va[:], va[:], mu2[:])
    S.activation(va[:], va[:], AF.Sqrt, bias=epsc[0:1, 0:1])
    V.reciprocal_approx_fast(va[:], va[:])
    rm = pool.tile([1, TOK], F32, name="ln_rm")
    V.tensor_mul(rm[:], va[:], mu[:])
    rstd_bf = pool.tile([1, TOK], BF16, name="ln_rstd_bf")
    V.tensor_copy(rstd_bf[:], va[:])
    rm_bf = pool.tile([1, TOK], BF16, name="ln_rm_bf")
    V.tensor_copy(rm_bf[:], rm[:])

    for ci in range(8):
        for nt in range(2):
            a_ps = psum.tile([P, 512], F32, name="lnA_ps")
            b_ps = psum.tile([P, 512], F32, name="lnB_ps")
            tsl = slice(nt * 512, (nt + 1) * 512)
            mm(a_ps[:], sp_row[0:1, ci * P:(ci + 1) * P], rstd_bf[0:1, tsl],
               start=True, stop=True)
            mm(b_ps[:], sp_row[0:1, ci * P:(ci + 1) * P], rm_bf[0:1, tsl],
               start=True, stop=True)
            sl = slice(ci * TOK + nt * 512, ci * TOK + (nt + 1) * 512)
            t1 = tmp.tile([P, 512], BF16, name="ln_t1")
            V.tensor_mul(t1[:], xc[:, sl], a_ps[:])
            V.scalar_tensor_tensor(out_bf[:, sl], t1[:],
                                   sh_col[:, ci:ci + 1], b_ps[:],
                                   ALU.add, ALU.subtract)
        if block_done is not None and ci % 2 == 1:
            block_done(ci // 2)


def qkv_phase(nc, tc, io, dbg, slab, tmp, rhsp,
              h1_all, a2a_in, mm, mm1,
              Cb, Sb, sel2, sel2T, ones128, ones65, qb_col, vb128, perm_sb,
              ident_sb, scale_sb):
    V = nc.vector
    S = nc.scalar
    dma = nc.sync.dma_start

    w_sb = slab.tile([P, 8 * 384], BF16, name="w_sb")
    dma(out=w_sb[:].rearrange("p (blk c) -> p blk c", blk=8),
        in_=io["w_qkv"].ap().rearrange("(blk p) c -> p blk c", p=P))

    qn = slab.tile([P, B * L], BF16, name="qn")      # (128, 8192)
    kn = slab.tile([P, B * L], BF16, name="kn")
    v_sb = slab.tile([P, B * 16 * 2 * 65], BF16, name="v_sb")
    V.memset(v_sb[:].rearrange("p (blk c) -> p blk c", c=65)[:, :, 64:65], 1.0)
    invk_raw = slab.tile([P, P], F32, name="invk_raw")

    qkv_loop(nc, tc, io, slab, tmp, rhsp, h1_all, mm, mm1,
             Cb, Sb, sel2, sel2T, ones128, qb_col, perm_sb, ident_sb,
             scale_sb, w_sb, qn, kn, v_sb, invk_raw)

    invk = slab.tile([P, P], F32, name="invk")
    S.activation(invk[:], invk_raw[:], AF.Sqrt)
    V.tensor_scalar_max(invk[:], invk[:], 1e-12)
    V.reciprocal_approx_fast(invk[:], invk[:])

    for name, t in (("qn", qn), ("kn", kn), ("v_sb", v_sb)):
        if name in dbg:
            dma(out=dbg[name][:], in_=t[:])

    attention(nc, tc, dbg, slab, tmp, a2a_in, mm, mm1,
              ones65, vb128, qn, kn, v_sb, invk)


def qkv_loop(nc, tc, io, slab, tmp, rhsp, h1_all, mm, mm1,
             Cb, Sb, sel2, sel2T, ones128, qb_col, perm_sb, ident_sb,
             scale_sb, w_sb, qn, kn, v_sb, invk_raw):
    V = nc.vector
    S = nc.scalar
    dma = nc.sync.dma_start
    psum_cm = tc.tile_pool(name="qkvpsum", bufs=1, space="PSUM")
    psum = psum_cm.__enter__()

    def process_q(q_ps, blk, nt):
        gsl = slice(blk * TOK + nt * 512, blk * TOK + (nt + 1) * 512)
        cpos = (blk % 2) * TOK + nt * 512
        csl = slice(cpos, cpos + 512)
        qb = tmp.tile([P, 512], BF16, name="qb")
        V.tensor_scalar_add(qb[:], q_ps[:], qb_col[:, 0:1])
        q2 = tmp.tile([P, 512], BF16, name="q2")
        V.tensor_mul(q2[:], qb[:], qb[:])
        sq_ps = psum.tile([2, 512], F32, name="sq_ps", tag="red")
        mm1(sq_ps[:], sel2[:], q2[:])
        qsd = tmp.tile([2, 512], F32, name="qsd")
        S.activation(qsd[:], sq_ps[:], AF.Sqrt)
        V.tensor_scalar_max(qsd[:], qsd[:], 1e-12)
        iq = tmp.tile([2, 512], F32, name="iq")
        V.reciprocal_approx_fast(iq[:], qsd[:])
        iq_bf = tmp.tile([2, 512], BF16, name="iq_bf")
        V.tensor_scalar_mul(iq_bf[:], iq[:], scale_sb[:, 0:1])
        swp_ps = psum.tile([P, 512], F32, name="swp_ps", tag="bcast")
        mm1(swp_ps[:], perm_sb[:], qb[:])
        t1 = tmp.tile([P, 512], BF16, name="rope_t1")
        t2 = tmp.tile([P, 512], BF16, name="rope_t2")
        V.tensor_mul(t1[:], qb[:], Cb[:, csl])
        V.tensor_mul(t2[:], swp_ps[:], Sb[:, csl])
        qr = tmp.tile([P, 512], BF16, name="qr")
        V.tensor_add(qr[:], t1[:], t2[:])
        ib_ps = psum.tile([P, 512], F32, name="ib_ps", tag="bcast")
        mm1(ib_ps[:], sel2T[:], iq_bf[:])
        V.tensor_mul(qn[:, gsl], qr[:], ib_ps[:])

    def process_k(k_ps, blk, nt):
        gsl = slice(blk * TOK + nt * 512, blk * TOK + (nt + 1) * 512)
        cpos = (blk % 2) * TOK + nt * 512
        csl = slice(cpos, cpos + 512)
        b_idx = blk // 2
        kb = tmp.tile([P, 512], BF16, name="kb")
        V.tensor_copy(kb[:], k_ps[:])
        k2 = tmp.tile([P, 512], BF16, name="k2")
        V.tensor_mul(k2[:], kb[:], kb[:])
        ks_ps = psum.tile([P, 8], F32, name="ks_ps", tag="red")
        for hh in range(2):
            for t4 in range(4):
                mm1(ks_ps[:, hh * 4 + t4:hh * 4 + t4 + 1],
                    k2[hh * 64:(hh + 1) * 64, t4 * 128:(t4 + 1) * 128],
                    ones128[hh * 64:(hh + 1) * 64, 0:1])
        kt0 = (blk % 2) * 8 + nt * 4
        base = (b_idx * 16 + kt0) * 2
        V.tensor_copy(
            invk_raw[:, base:base + 8]
            .rearrange("p (t4 h) -> p h t4", h=2),
            ks_ps[:].rearrange("p (h t4) -> p h t4", h=2))
        kswp_ps = psum.tile([P, 512], F32, name="kswp_ps", tag="bcast")
        mm1(kswp_ps[:], perm_sb[:], kb[:])
        t1 = tmp.tile([P, 512], BF16, name="rope_t1")
        t2 = tmp.tile([P, 512], BF16, name="rope_t2")
        V.tensor_mul(t1[:], kb[:], Cb[:, csl])
        V.tensor_mul(t2[:], kswp_ps[:], Sb[:, csl])
        V.tensor_add(kn[:, gsl], t1[:], t2[:])

    for blk in range(8):
        b_idx = blk // 2
        rhs = {}
        for nt in range(2):
            for ci in range(8):
                r = rhsp.tile([P, 512], BF16, name="h1r")
                dma(out=r[:], in_=h1_all[ci // 2][blk,
                                                  (ci % 2) * P:(ci % 2 + 1) * P,
                                                  nt * 512:(nt + 1) * 512])
                rhs[(nt, ci)] = r

        # q/k accumulation; lhsT reused across nt (one LDW per 2 matmuls)
        acc = {}
        for wname in ("q", "k"):
            for nt in range(2):
                acc[(wname, nt)] = psum.tile(
                    [P, 512], F32, name=f"{wname}{nt}_ps",
                    tag=f"{wname}{nt}_ps")
        for ci in range(8):
            for w_off, wname in ((0, "q"), (128, "k")):
                wt = w_sb[:, ci * 384 + w_off:ci * 384 + w_off + 128]
                for nt in range(2):
                    mm(acc[(wname, nt)][:], wt, rhs[(nt, ci)][:],
                       start=(ci == 0), stop=(ci == 7))
        for nt in range(2):
            process_q(acc[("q", nt)], blk, nt)
            process_k(acc[("k", nt)], blk, nt)

        # v: co-major matmul then PE transpose to token-major
        for nt in range(2):
            v_ps = psum.tile([P, 512], F32, name="v_ps", tag="vtp", bufs=2)
            for ci in range(8):
                mm(v_ps[:], w_sb[:, ci * 384 + 256:ci * 384 + 384],
                   rhs[(nt, ci)][:], start=(ci == 0), stop=(ci == 7))
            vco = tmp.tile([P, 512], BF16, name="vco")
            V.tensor_copy(vco[:], v_ps[:])
            kt0 = (blk % 2) * 8 + nt * 4
            for t4 in range(4):
                tp_ps = psum.tile([P, P], BF16, name="tp_ps", tag="vtp",
                                  bufs=2)
                nc.tensor.transpose(tp_ps[:], vco[:, t4 * 128:(t4 + 1) * 128],
                                    ident_sb[:])
                kt = kt0 + t4
                vbase = (b_idx * 16 + kt) * 2 * 65
                V.tensor_copy(
                    v_sb[:, vbase:vbase + 130]
                    .rearrange("p (h c) -> p h c", h=2)[:, :, 0:64],
                    tp_ps[:].rearrange("p (h c) -> p h c", h=2))
    psum_cm.__exit__(None, None, None)


def attention(nc, tc, dbg, slab, tmp, a2a_in, mm, mm1,
              ones65, vb128, qn, kn, v_sb, invk):
    V = nc.vector
    S = nc.scalar
    dma = nc.sync.dma_start
    psum_cm = tc.tile_pool(name="atnpsum", bufs=1, space="PSUM")
    psum = psum_cm.__enter__()
    attn = slab.tile([P, B * L], BF16, name="attn")

    pending = []

    # eviction: per (b, qh, hh, j) the pv (65, 512) -> attn rows hh*64..
    def evict(b2, q2, items):
        for (hh, j, pvall) in items:
            rec = tmp.tile([65, 512], F32, name="rec")
            V.reciprocal_approx_fast(rec[:], pvall[:])
            rec_bf = tmp.tile([65, 512], BF16, name="rec_bf")
            V.tensor_copy(rec_bf[64:65, :], rec[64:65, :])
            rb_ps = psum.tile([P, 1024], F32, name="rb_ps",
                              tag="s_h0")
            mm(rb_ps[hh * 64:hh * 64 + 64, 0:512], ones65[64:65, :],
               rec_bf[64:65, :], start=True, stop=True)
            tm = tmp.tile([P, 512], BF16, name="tm")
            V.tensor_mul(tm[hh * 64:(hh + 1) * 64, :], pvall[0:64, :],
                         rb_ps[hh * 64:hh * 64 + 64, 0:512])
            col = b2 * L + q2 * 1024 + j * 512
            V.tensor_scalar_add(attn[hh * 64:(hh + 1) * 64, col:col + 512],
                                tm[hh * 64:(hh + 1) * 64, :],
                                vb128[hh * 64:(hh + 1) * 64, 0:1])

    for b_idx in range(B):
        for qh in range(2):
            pv = {}
            for hh in range(2):
                for j in range(2):
                    pv[(hh, j)] = psum.tile(
                        [65, 512], F32, name=f"pv{hh}{j}", tag=f"pv{hh}{j}")

            def drain(item):
                ktd, es = item
                for hh in range(2):
                    vb = ((b_idx * 16 + ktd) * 2 + hh) * 65
                    for j in range(2):
                        mm(pv[(hh, j)][:], v_sb[:, vb:vb + 65],
                           es[hh][:, j * 512:(j + 1) * 512],
                           start=(ktd == 0), stop=(ktd == 15))

            pend = []
            for kt in range(16):
                ksl = slice(b_idx * L + kt * 128, b_idx * L + (kt + 1) * 128)
                sh = []
                for hh in range(2):
                    s_h = psum.tile([P, 1024], F32, name=f"s_h{hh}",
                                    tag=f"s_h{hh}")
                    sh.append(s_h)
                # interleave heads so row-groups 0-63 / 64-127 overlap in PE
                for j in range(2):
                    qsl = slice(b_idx * L + qh * 1024 + j * 512,
                                b_idx * L + qh * 1024 + (j + 1) * 512)
                    for hh in range(2):
                        hs = slice(hh * 64, (hh + 1) * 64)
                        mm1(sh[hh][:, j * 512:(j + 1) * 512],
                            kn[hs, ksl], qn[hs, qsl])
                es = []
                for hh in range(2):
                    e_bf = tmp.tile([P, 1024], BF16, name="e_bf", bufs=6)
                    ikcol = (b_idx * 16 + kt) * 2 + hh
                    S.activation(e_bf[:], sh[hh][:], AF.Exp,
                                 scale=invk[:, ikcol:ikcol + 1])
                    es.append(e_bf)
                pend.append((kt, es))
                if len(pend) > 2:
                    drain(pend.pop(0))
                if kt in (6, 8, 10, 12) and pending:
                    b2, q2, items = pending[0]
                    evict(b2, q2, [items.pop(0)])
                    if not items:
                        pending.pop(0)
            for item in pend:
                drain(item)

            items = []
            for hh in range(2):
                for j in range(2):
                    pvall = tmp.tile([65, 512], F32, name="pvall", bufs=8)
                    V.tensor_copy(pvall[:], pv[(hh, j)][:])
                    items.append((hh, j, pvall))
            pending.append((b_idx, qh, items))
    while pending:
        b2, q2, items = pending.pop(0)
        evict(b2, q2, items)

    psum_cm.__exit__(None, None, None)
    if "attn" in dbg:
        dma(out=dbg["attn"][:], in_=attn[:])
    dma(out=a2a_in[:].rearrange("blk p t -> p blk t"),
        in_=attn[:].rearrange("p (blk t) -> p blk t", blk=8))


# ---------------------------------------------------------------------------
# host-side input preparation
# ---------------------------------------------------------------------------

_PERM = np.concatenate([np.arange(0, HD, 2), np.arange(1, HD, 2)])  # re|im


def _perm_matrix():
    """(128,128) with entry (swap(m), m) = 1; swap exchanges the re (0:32)
    and im (32:64) halves of each 64-row head slice."""
    pm = np.zeros((P, P), np.float32)
    for m in range(P):
        base = (m // 64) * 64
        r = m - base
        sw = base + (r + 32) % 64
        pm[sw, m] = 1.0
    return pm


def prep_in_maps(inputs):
    import ml_dtypes
    bf = lambda a: np.ascontiguousarray(a).astype(ml_dtypes.bfloat16)
    f32 = lambda a: np.ascontiguousarray(np.asarray(a, dtype=np.float32))

    x = np.asarray(inputs["x"], np.float32)
    cond = np.asarray(inputs["cond_BD"], np.float32)
    W_qkv = np.asarray(inputs["W_qkv"], np.float32)
    q_bias = np.asarray(inputs["q_bias"], np.float32)
    v_bias = np.asarray(inputs["v_bias"], np.float32)
    sml = np.asarray(inputs["scale_mul_log"], np.float32).reshape(H)
    W_proj = np.asarray(inputs["W_proj"], np.float32)
    b_proj = np.asarray(inputs["b_proj"], np.float32)
    W_fc1 = np.asarray(inputs["W_fc1"], np.float32)
    b_fc1 = np.asarray(inputs["b_fc1"], np.float32)
    W_fc2 = np.asarray(inputs["W_fc2"], np.float32)
    b_fc2 = np.asarray(inputs["b_fc2"], np.float32)
    W_ada = np.asarray(inputs["W_ada"], np.float32)
    b_ada = np.asarray(inputs["b_ada"], np.float32)
    fc = np.asarray(inputs["freqs_cos"], np.float32)
    fs = np.asarray(inputs["freqs_sin"], np.float32)

    cosT = f32(fc.T)   # (32, L)
    sinT = f32(fs.T)
    w_projT = bf(W_proj.T)
    w_fc1T = bf(W_fc1.T)
    w_fc2T = bf(W_fc2.T)
    b_proj_c = f32(b_proj.reshape(8, P).T)
    b_fc1_c = f32(b_fc1.reshape(32, P).T)
    b_fc2_c = f32(b_fc2.reshape(8, P).T)
    w_adaT = W_ada.T  # (1024, 6144)

    in_maps = []
    for m in range(N_CORES):
        b_own, pm = m // 2, m % 2
        h0, h1 = 2 * m, 2 * m + 1
        cols = []
        for h in (h0, h1):
            cols.append(W_qkv[h * HD + _PERM, :])          # q rows, permuted
        for h in (h0, h1):
            cols.append(W_qkv[C + h * HD + _PERM, :])      # k rows, permuted
        for h in (h0, h1):
            cols.append(W_qkv[2 * C + h * HD:2 * C + (h + 1) * HD, :])  # v
        w_qkv_m = bf(np.concatenate(cols, axis=0).T)       # (1024, 384)
        qkv_b_m = np.concatenate([
            q_bias[h0 * HD + _PERM], q_bias[h1 * HD + _PERM],
            np.zeros(P, np.float32),
            v_bias[h0 * HD:(h0 + 1) * HD], v_bias[h1 * HD:(h1 + 1) * HD],
        ]).reshape(384, 1)

        vb2 = np.stack([v_bias[h0 * HD:(h0 + 1) * HD],
                        v_bias[h1 * HD:(h1 + 1) * HD]], axis=1)
        vb128 = np.concatenate([v_bias[h0 * HD:(h0 + 1) * HD],
                                v_bias[h1 * HD:(h1 + 1) * HD]]).reshape(P, 1)
        xm = x[b_own, pm * TOK:(pm + 1) * TOK, :]          # (1024, 1024)
        identm = np.eye(P, dtype=np.float32)
        s2t = np.zeros((2, P), np.float32)
        s2t[0, 0:64] = 1.0
        s2t[1, 64:128] = 1.0
        in_maps.append({
            "perm_m": bf(_perm_matrix()),
            "sel2t": bf(s2t),
            "ident": bf(identm),
            "vb2": f32(vb2),
            "vb128": f32(vb128),
            "xT": f32(xm.T),
            "condT": f32(cond[b_own].reshape(C, 1)),
            "w_ada": bf(w_adaT[:, pm * 3 * C:(pm + 1) * 3 * C]),
            "b_ada_r": f32(b_ada[pm * 3 * C:(pm + 1) * 3 * C].reshape(1, -1)),
            "w_qkv": w_qkv_m,
            "qkv_b": f32(qkv_b_m),
            "scale_log": f32(sml[[h0, h1]].reshape(2, 1)),
            "cosT": cosT, "sinT": sinT,
            "w_proj": w_projT, "b_proj_c": b_proj_c,
            "w_fc1": w_fc1T, "b_fc1_c": b_fc1_c,
            "w_fc2": w_fc2T, "b_fc2_c": b_fc2_c,
        })
    return in_maps


_NC_CACHE = {}


def _get_nc(debug_outputs=()):
    key = tuple(sorted(debug_outputs))
    if key not in _NC_CACHE:
        _NC_CACHE[key] = build_nc(debug_outputs)
    return _NC_CACHE[key]


def run(inputs, debug_outputs=(), trace=False):
    nc = _get_nc(debug_outputs)
    in_maps = prep_in_maps(inputs)
    res = run_bass_kernel_spmd(nc, in_maps, core_ids=list(range(N_CORES)),
                               trace=trace)
    out = np.empty((B, L, C), np.float32)
    for m in range(N_CORES):
        b_own, pm = m // 2, m % 2
        out[b_own, pm * TOK:(pm + 1) * TOK, :] = res.results[m]["outT"].T
    return out, res


def kernel(**inputs):
    out, _ = run(inputs)
    return out


# revision 33
# speedup vs baseline: 1.1158x; 1.1158x over previous
"""Trainium2 Bass kernel for nn_AdaLNSelfAttn_RoPE (B=4, L=2048, C=1024, H=16).

Sharding across 8 NeuronCores (one chip):
  - Tokens flattened (B*L = 8192) and sharded 1024/core; each core owns half of
    one batch.  LayerNorms, AdaLN modulation, proj, and the FFN run on the
    token shard (sequence parallel).
  - QKV + attention are head-sharded: core m owns heads {2m, 2m+1}.  The
    modulated hidden h1 is AllGather'ed (bf16) so every core computes QKV for
    its two heads over all tokens.  Attention outputs return to token shards
    via AllToAll.
  - AdaLN (silu+linear) is column-sharded across pairs of cores (both cores of
    a pair own the same batch) with a 2-wide AllGather.
  - Activations are channel-major (C on partitions) so matmuls chain without
    transposes; per-token statistics (LN mean/rstd, softmax rowsums, q-norms)
    are reduced across partitions with ones-matmuls and broadcast back with
    outer-product matmuls.
  - Matmuls in bf16 (fp32 PSUM accumulation); statistics in fp32.
  - Softmax skips max-subtraction (cosine attention bounds scores to [-4,4]).
    The softmax denominator rides as a 65th output column of the PV matmul;
    the k-side 1/||k|| rides as the per-partition scale of the exp()
    activation.
  - RoPE pairs are pre-permuted on the host (re parts in rows 0-31 of each
    head slice, im parts in rows 32-63) so rotation is four 32-row block ops.
"""

import numpy as np

import concourse.bass as bass
import concourse.bacc as bacc
import concourse.mybir as mybir
import concourse.tile as tile
from concourse.bass_utils import run_bass_kernel_spmd

F32 = mybir.dt.float32
BF16 = mybir.dt.bfloat16
AF = mybir.ActivationFunctionType
ALU = mybir.AluOpType

B, L, C, H = 4, 2048, 1024, 16
HD = C // H          # 64
N_CORES = 8
TOK = (B * L) // N_CORES   # 1024 tokens per core
MAX_SCALE = float(np.log(100.0))
LN_EPS = 1e-6
P = 128


def build_nc(debug_outputs=()):
    nc = bacc.Bacc("TRN2", target_bir_lowering=False, debug=False,
                   num_devices=N_CORES)

    dt = nc.dram_tensor
    io = {}
    io["xT"] = dt("xT", [C, TOK], F32, kind="ExternalInput")
    io["condT"] = dt("condT", [C, 1], F32, kind="ExternalInput")
    io["w_ada"] = dt("w_ada", [C, 3 * C], BF16, kind="ExternalInput")
    io["b_ada_r"] = dt("b_ada_r", [1, 3 * C], F32, kind="ExternalInput")
    io["w_qkv"] = dt("w_qkv", [C, 384], BF16, kind="ExternalInput")
    io["qkv_b"] = dt("qkv_b", [384, 1], F32, kind="ExternalInput")
    io["perm_m"] = dt("perm_m", [P, P], BF16, kind="ExternalInput")
    io["sel2t"] = dt("sel2t", [2, P], BF16, kind="ExternalInput")
    io["ident"] = dt("ident", [P, P], BF16, kind="ExternalInput")
    io["vb2"] = dt("vb2", [64, 2], F32, kind="ExternalInput")
    io["vb128"] = dt("vb128", [P, 1], F32, kind="ExternalInput")
    io["scale_log"] = dt("scale_log", [2, 1], F32, kind="ExternalInput")
    io["cosT"] = dt("cosT", [32, L], F32, kind="ExternalInput")
    io["sinT"] = dt("sinT", [32, L], F32, kind="ExternalInput")
    io["w_proj"] = dt("w_proj", [C, C], BF16, kind="ExternalInput")
    io["b_proj_c"] = dt("b_proj_c", [P, 8], F32, kind="ExternalInput")
    io["w_fc1"] = dt("w_fc1", [C, 4 * C], BF16, kind="ExternalInput")
    io["b_fc1_c"] = dt("b_fc1_c", [P, 32], F32, kind="ExternalInput")
    io["w_fc2"] = dt("w_fc2", [4 * C, C], BF16, kind="ExternalInput")
    io["b_fc2_c"] = dt("b_fc2_c", [P, 8], F32, kind="ExternalInput")
    io["outT"] = dt("outT", [C, TOK], F32, kind="ExternalOutput")

    dbg = {}
    for name, shape, dtp in [
        ("h1_all", [8, C, TOK], BF16),
        ("qn", [P, B * L], BF16),
        ("kn", [P, B * L], BF16),
        ("v_sb", [P, B * 16 * 2 * 65], BF16),
        ("attn", [P, B * L], BF16),
        ("a2a_out", [8, P, TOK], BF16),
        ("x2", [P, 8 * TOK], F32),
        ("ada_all", [2, 3 * C], F32),
    ]:
        if name in debug_outputs:
            dbg[name] = dt("dbg_" + name, shape, dtp, kind="ExternalOutput")

    with tile.TileContext(nc) as tc:
        _body(nc, tc, io, dbg)
    nc.compile()
    return nc


def _body(nc, tc, io, dbg):
    mm = nc.tensor.matmul
    V = nc.vector
    S = nc.scalar
    dma = nc.sync.dma_start

    def mm1(out, lhsT, rhs):
        mm(out, lhsT, rhs, start=True, stop=True)

    with tc.tile_pool(name="dram", bufs=1, space="DRAM") as dram, \
         tc.tile_pool(name="const", bufs=1) as const, \
         tc.tile_pool(name="outer", bufs=1) as outer:

        # DRAM bounce buffers for collectives
        ada_in = dram.tile([1, 3 * C], F32, name="ada_in")
        ada_all = dram.tile([2, 3 * C], F32, name="ada_all")
        h1_in = dram.tile([TOK, C], BF16, name="h1_in")
        h1_all = [dram.tile([8, C, TOK], BF16, addr_space="Shared",
                            name="h1_all0")]
        a2a_in = dram.tile([8, P, TOK], BF16, name="a2a_in")
        a2a_out = dram.tile([8, P, TOK], BF16, name="a2a_out")

        # ------------------------------------------------------- constants
        ones128 = const.tile([P, 1], BF16, name="ones128")
        V.memset(ones128[:], 1.0)
        ones64c = const.tile([64, 1], BF16, name="ones64c")
        V.memset(ones64c[:], 1.0)
        ones1x64 = const.tile([1, 64], BF16, name="ones1x64")
        V.memset(ones1x64[:], 1.0)
        sel2 = const.tile([P, 2], BF16, name="sel2")     # head indicator cols
        V.memset(sel2[:], 0.0)
        V.memset(sel2[0:64, 0:1], 1.0)
        V.memset(sel2[64:128, 1:2], 1.0)
        sel2T = const.tile([2, P], BF16, name="sel2T")   # head indicator rows
        dma(out=sel2T[:], in_=io["sel2t"][:, :])

        # rope cos (128, 2048) bf16: 4 vertical copies of (32, 2048); and
        # sign-baked sin: rows [0:32]=-sin [32:64]=+sin [64:96]=-sin [96:]=+sin
        Cb = const.tile([P, L], BF16, name="Cb")
        Sb = const.tile([P, L], BF16, name="Sb")
        with tc.tile_pool(name="cs_pool", bufs=1) as csp:
            cs_f = csp.tile([P, L], F32, name="cs_f")
            for j in range(4):
                dma(out=cs_f[32 * j:32 * (j + 1), :], in_=io["cosT"][:, :])
            V.tensor_copy(Cb[:], cs_f[:])
            sn_f = csp.tile([P, L], F32, name="sn_f")
            for j in range(4):
                dma(out=sn_f[32 * j:32 * (j + 1), :], in_=io["sinT"][:, :])
            for j in range(4):
                sgn = -1.0 if j % 2 == 0 else 1.0
                S.activation(Sb[32 * j:32 * (j + 1), :],
                             sn_f[32 * j:32 * (j + 1), :], AF.Copy, scale=sgn)
        perm_sb = const.tile([P, P], BF16, name="perm_sb")
        dma(out=perm_sb[:], in_=io["perm_m"][:, :])
        ident_sb = const.tile([P, P], BF16, name="ident_sb")
        dma(out=ident_sb[:], in_=io["ident"][:, :])
        vb2 = const.tile([64, 2], F32, name="vb2")
        dma(out=vb2[:], in_=io["vb2"][:, :])
        vb128 = const.tile([P, 1], F32, name="vb128")
        dma(out=vb128[:], in_=io["vb128"][:, :])
        ones65 = const.tile([65, 64], BF16, name="ones65")
        V.memset(ones65[:], 1.0)

        # bias columns
        qb_col = const.tile([P, 3], F32, name="qb_col")
        dma(out=qb_col[:], in_=io["qkv_b"].ap().rearrange("(m p) 1 -> p m", p=P))
        bproj = const.tile([P, 8], F32, name="bproj")
        dma(out=bproj[:], in_=io["b_proj_c"][:, :])
        bfc1 = const.tile([P, 32], F32, name="bfc1")
        dma(out=bfc1[:], in_=io["b_fc1_c"][:, :])
        bfc2 = const.tile([P, 8], F32, name="bfc2")
        dma(out=bfc2[:], in_=io["b_fc2_c"][:, :])
        bada = const.tile([1, 3 * C], F32, name="bada")
        dma(out=bada[:], in_=io["b_ada_r"][:, :])
        epsc = const.tile([1, 1], F32, name="epsc")
        V.memset(epsc[:], LN_EPS)

        # scale_mul = exp(min(scale_log, MAX_SCALE))
        scale_sb = const.tile([2, 1], F32, name="scale_sb")
        with tc.tile_pool(name="scp", bufs=1) as scp:
            sc_raw = scp.tile([2, 1], F32, name="sc_raw")
            dma(out=sc_raw[:], in_=io["scale_log"][:, :])
            sc_min = scp.tile([2, 1], F32, name="sc_min")
            V.tensor_scalar_min(sc_min[:], sc_raw[:], MAX_SCALE)
            S.activation(scale_sb[:], sc_min[:], AF.Exp)

        # --------------------------------------------------- phase 0: adaLN
        ada_phase(nc, tc, io, dram, const, ada_in, ada_all)

        g1c = ada_col(nc, const, ada_all, 0, "g1c")
        g2c = ada_col(nc, const, ada_all, 1, "g2c")
        sh1c = ada_col(nc, const, ada_all, 4, "sh1c")
        sh2c = ada_col(nc, const, ada_all, 5, "sh2c")
        s1p = ada_srow(nc, tc, const, ada_all, 2, "s1p")
        s2p = ada_srow(nc, tc, const, ada_all, 3, "s2p")

        if "ada_all" in dbg:
            dma(out=dbg["ada_all"][:], in_=ada_all[:])

        # ------------------------------------------------------ LN1 -> AG h1
        with tc.tile_pool(name="ln1pool", bufs=1) as lp, \
             tc.tile_pool(name="ln1psum", bufs=2, space="PSUM") as lps, \
             tc.tile_pool(name="ln1tmp", bufs=3) as ltmp:
            xT_sb = lp.tile([P, 8 * TOK], F32, name="xT_sb")
            dma(out=xT_sb[:].rearrange("p (blk t) -> p blk t", blk=8),
                in_=io["xT"].ap().rearrange("(blk p) t -> p blk t", p=P))
            h1_bf = lp.tile([P, 8 * TOK], BF16, name="h1_bf")

            def h1_chunk_out(ci2):
                # stage channel blocks as they complete; one AG at the end
                csl = slice(2 * ci2 * P, 2 * (ci2 + 1) * P)
                dma(out=h1_in[csl, :].rearrange("(blk p) t -> p blk t", p=P),
                    in_=h1_bf[:, 2 * ci2 * TOK:2 * (ci2 + 1) * TOK]
                    .rearrange("p (blk t) -> p blk t", blk=2))

            layer_norm(nc, lp, lps, ltmp, xT_sb, s1p, sh1c, h1_bf,
                       ones128, epsc, mm, block_done=h1_chunk_out)
        nc.gpsimd.collective_compute(
            "AllGather", ALU.bypass,
            replica_groups=[list(range(N_CORES))],
            ins=[h1_in[:].opt()], outs=[h1_all[0][:].opt()])
        if "h1_all" in dbg:
            dma(out=dbg["h1_all"][:], in_=h1_all[0][:])

        # ------------------------------------- phase 2+3: qkv, rope, attention
        with tc.tile_pool(name="atslab", bufs=1) as ats, \
             tc.tile_pool(name="attmp", bufs=2) as atmp, \
             tc.tile_pool(name="atrhs", bufs=18) as arhs:
            qkv_phase(nc, tc, io, dbg, ats, atmp, arhs,
                      h1_all, a2a_in, mm, mm1,
                      Cb, Sb, sel2, sel2T, ones128, ones65,
                      qb_col, vb128, perm_sb, ident_sb, scale_sb)
        nc.gpsimd.collective_compute(
            "AllToAll", ALU.bypass,
            replica_groups=[list(range(N_CORES))],
            ins=[a2a_in[:].opt()], outs=[a2a_out[:].opt()])
        if "a2a_out" in dbg:
            dma(out=dbg["a2a_out"][:], in_=a2a_out[:])

        # ----------------------------------------- phase 4: proj + residual
        x2p_cm = tc.tile_pool(name="x2pool", bufs=1)
        x2p = x2p_cm.__enter__()
        x2_sb = x2p.tile([P, 8 * TOK], F32, name="x2_sb")
        with tc.tile_pool(name="pjpool", bufs=1) as pjp, \
             tc.tile_pool(name="pjpsum", bufs=2, space="PSUM") as pjps, \
             tc.tile_pool(name="pjtmp", bufs=3) as ptmp:
            wp_sb = pjp.tile([P, 8 * C], BF16, name="wp_sb")
            dma(out=wp_sb[:].rearrange("p (blk c) -> p blk c", blk=8),
                in_=io["w_proj"].ap().rearrange("(blk p) c -> p blk c", p=P))
            prhs = []
            for r in range(8):
                for nt in range(2):
                    t = pjp.tile([P, 512], BF16, name=f"prhs_{r}_{nt}",
                                 tag="prhs", bufs=16)
                    dma(out=t[:], in_=a2a_out[r, :, nt * 512:(nt + 1) * 512])
                    prhs.append(t)
            for cot in range(8):
                xres = ptmp.tile([P, 2 * 512], F32, name="xres")
                dma(out=xres[:],
                    in_=io["xT"][cot * P:(cot + 1) * P, :])
                pp = [pjps.tile([P, 512], F32, name=f"p_ps{nt}",
                                tag=f"p_ps{nt}") for nt in range(2)]
                for r in range(8):
                    wt = wp_sb[:, r * C + cot * P:r * C + (cot + 1) * P]
                    for nt in range(2):
                        mm(pp[nt][:], wt, prhs[r * 2 + nt][:],
                           start=(r == 0), stop=(r == 7))
                for nt in range(2):
                    t1 = ptmp.tile([P, 512], F32, name="pj_t1")
                    V.tensor_scalar(t1[:], pp[nt][:], bproj[:, cot:cot + 1],
                                    g1c[:, cot:cot + 1], ALU.add, ALU.mult)
                    sl = slice(cot * TOK + nt * 512, cot * TOK + (nt + 1) * 512)
                    V.tensor_add(x2_sb[:, sl], t1[:],
                                 xres[:, nt * 512:(nt + 1) * 512])
        if "x2" in dbg:
            dma(out=dbg["x2"][:], in_=x2_sb[:])

        # ------------------------------------------------- phase 5-7: LN2+FFN
        with tc.tile_pool(name="ffnpool", bufs=1) as fp, \
             tc.tile_pool(name="ffntmp", bufs=3) as ftmp:
            h2_bf = fp.tile([P, 8 * TOK], BF16, name="h2_bf")
            with tc.tile_pool(name="ln2pool", bufs=1) as lp2, \
                 tc.tile_pool(name="ln2psum", bufs=2, space="PSUM") as lps2:
                layer_norm(nc, lp2, lps2, ftmp, x2_sb, s2p, sh2c, h2_bf,
                           ones128, epsc, mm)

            fw_cm = tc.tile_pool(name="ffnw", bufs=2)
            fw = fw_cm.__enter__()
            fps_cm = tc.tile_pool(name="ffnpsum", bufs=4, space="PSUM")
            fps = fps_cm.__enter__()
            hact = fp.tile([P, 32 * TOK], BF16, name="hact")
            for cot in range(32):
                w1 = fw.tile([P, 8 * P], BF16, name="w1")
                dma(out=w1[:].rearrange("p (blk c) -> p blk c", blk=8),
                    in_=io["w_fc1"].ap()[:, cot * P:(cot + 1) * P]
                    .rearrange("(blk p) c -> p blk c", p=P))
                fpp = [fps.tile([P, 512], F32, name=f"f_ps{nt}",
                                tag=f"f_ps{nt}", bufs=2) for nt in range(2)]
                for ci in range(8):
                    wt = w1[:, ci * P:(ci + 1) * P]
                    for nt in range(2):
                        mm(fpp[nt][:], wt,
                           h2_bf[:, ci * TOK + nt * 512:
                                 ci * TOK + (nt + 1) * 512],
                           start=(ci == 0), stop=(ci == 7))
                for nt in range(2):
                    S.activation(
                        hact[:, cot * TOK + nt * 512:cot * TOK + (nt + 1) * 512],
                        fpp[nt][:], AF.Gelu_apprx_tanh,
                        bias=bfc1[:, cot:cot + 1])

            for cot in range(8):
                w2 = fw.tile([P, 32 * P], BF16, name="w2")
                dma(out=w2[:].rearrange("p (blk c) -> p blk c", blk=32),
                    in_=io["w_fc2"].ap()[:, cot * P:(cot + 1) * P]
                    .rearrange("(blk p) c -> p blk c", p=P))
                opp = [fps.tile([P, 512], F32, name=f"o_ps{nt}",
                                tag=f"o_ps{nt}", bufs=2) for nt in range(2)]
                for ci in range(32):
                    wt = w2[:, ci * P:(ci + 1) * P]
                    for nt in range(2):
                        mm(opp[nt][:], wt,
                           hact[:, ci * TOK + nt * 512:
                                 ci * TOK + (nt + 1) * 512],
                           start=(ci == 0), stop=(ci == 31))
                for nt in range(2):
                    t1 = ftmp.tile([P, 512], F32, name="o_t1")
                    V.tensor_scalar(t1[:], opp[nt][:], bfc2[:, cot:cot + 1],
                                    g2c[:, cot:cot + 1], ALU.add, ALU.mult)
                    ot = ftmp.tile([P, 512], F32, name="ot")
                    sl = slice(cot * TOK + nt * 512, cot * TOK + (nt + 1) * 512)
                    V.tensor_add(ot[:], t1[:], x2_sb[:, sl])
                    dma(out=io["outT"][cot * P:(cot + 1) * P,
                                       nt * 512:(nt + 1) * 512],
                        in_=ot[:])
            fps_cm.__exit__(None, None, None)
            fw_cm.__exit__(None, None, None)
        x2p_cm.__exit__(None, None, None)


def ada_phase(nc, tc, io, dram, const, ada_in, ada_all):
    """silu(cond) @ W_ada_slice.T + b_ada, pair-wise AllGather."""
    mm = nc.tensor.matmul
    V = nc.vector
    S = nc.scalar
    dma = nc.sync.dma_start
    cond_sb = const.tile([P, 8], F32, name="cond_sb")
    dma(out=cond_sb[:],
        in_=io["condT"].ap().rearrange("(blk p) 1 -> p blk", p=P))
    scond = const.tile([P, 8], BF16, name="scond")
    S.activation(scond[:], cond_sb[:], AF.Silu)
    bada = const.tile([1, 3 * C], F32, name="bada2")
    dma(out=bada[:], in_=io["b_ada_r"][:, :])

    with tc.tile_pool(name="adapool", bufs=1) as ap_, \
         tc.tile_pool(name="adapsum", bufs=2, space="PSUM") as aps, \
         tc.tile_pool(name="adarhs", bufs=4) as arp:
        ada_sb = ap_.tile([1, 3 * C], F32, name="ada_sb")
        for nt6 in range(6):
            a_ps = aps.tile([1, 512], F32, name="a_ps")
            for ci in range(8):
                wt = arp.tile([P, 512], BF16, name="wt_ada")
                dma(out=wt[:], in_=io["w_ada"][ci * P:(ci + 1) * P,
                                               nt6 * 512:(nt6 + 1) * 512])
                mm(a_ps[:], scond[:, ci:ci + 1], wt[:],
                   start=(ci == 0), stop=(ci == 7))
            V.tensor_add(ada_sb[0:1, nt6 * 512:(nt6 + 1) * 512], a_ps[:],
                         bada[0:1, nt6 * 512:(nt6 + 1) * 512])
        dma(out=ada_in[:], in_=ada_sb[:])
    nc.gpsimd.collective_compute(
        "AllGather", ALU.bypass,
        replica_groups=[[0, 1], [2, 3], [4, 5], [6, 7]],
        ins=[ada_in[:].opt()], outs=[ada_all[:].opt()])


def ada_col(nc, const, ada_all, vec, name):
    r, off = (vec * C) // (3 * C), (vec * C) % (3 * C)
    t = const.tile([P, 8], F32, name=name)
    nc.sync.dma_start(out=t[:], in_=ada_all[r:r + 1, off:off + C]
                      .rearrange("1 (blk p) -> p blk", p=P))
    return t


def ada_srow(nc, tc, const, ada_all, vec, name):
    """(1, 1024) bf16 row of (ada_vec + 1)."""
    r, off = (vec * C) // (3 * C), (vec * C) % (3 * C)
    t = const.tile([1, C], BF16, name=name)
    with tc.tile_pool(name=name + "_f", bufs=1) as p:
        raw = p.tile([1, C], F32, name=name + "_raw")
        nc.sync.dma_start(out=raw[:], in_=ada_all[r:r + 1, off:off + C])
        nc.vector.tensor_scalar_add(t[:], raw[:], 1.0)
    return t


def layer_norm(nc, pool, psum, tmp, x_sb, sp_row, sh_col, out_bf, ones128, epsc, mm,
               block_done=None):
    """x_sb (128, 8192) f32, channel-major blocks; out_bf same layout bf16:
    LN(x) * (s+1) + sh, statistics over the channel (partition x block) dim."""
    V = nc.vector
    S = nc.scalar
    xc = pool.tile([P, 8 * TOK], BF16, name="ln_xc")
    for ci in range(8):
        sl = slice(ci * TOK, (ci + 1) * TOK)
        V.tensor_copy(xc[:, sl], x_sb[:, sl])
    mu_ps = [psum.tile([1, 512], F32, name=f"mu_ps{nt}", tag=f"mu_ps{nt}",
                       bufs=1) for nt in range(2)]
    s2_ps = [psum.tile([1, 512], F32, name=f"s2_ps{nt}", tag=f"s2_ps{nt}",
                       bufs=1) for nt in range(2)]
    for ci in range(8):
        sl = slice(ci * TOK, (ci + 1) * TOK)
        xsq = tmp.tile([P, TOK], BF16, name="ln_xsq")
        V.tensor_mul(xsq[:], xc[:, sl], xc[:, sl])
        for nt in range(2):
            tsl = slice(ci * TOK + nt * 512, ci * TOK + (nt + 1) * 512)
            mm(mu_ps[nt][:], ones128[:], xc[:, tsl],
               start=(ci == 0), stop=(ci == 7))
            mm(s2_ps[nt][:], ones128[:], xsq[:, nt * 512:(nt + 1) * 512],
               start=(ci == 0), stop=(ci == 7))
    mu = pool.tile([1, TOK], F32, name="ln_mu")
    va = pool.tile([1, TOK], F32, name="ln_va")
    for nt in range(2):
        tsl = slice(nt * 512, (nt + 1) * 512)
        S.activation(mu[0:1, tsl], mu_ps[nt][:], AF.Copy, scale=1.0 / C)
        S.activation(va[0:1, tsl], s2_ps[nt][:], AF.Copy, scale=1.0 / C)
    # va := rstd = 1/sqrt(va - mu^2 + eps), in place
    mu2 = pool.tile([1, TOK], F32, name="ln_mu2")
    V.tensor_mul(mu2[:], mu[:], mu[:])
    V.tensor_sub(va[:], va[:], mu2[:])
    S.activation(va[:], va[:], AF.Sqrt, bias=epsc[0:1, 0:1])
    V.reciprocal_approx_fast(va[:], va[:])
    rm = pool.tile([1, TOK], F32, name="ln_rm")
    V.tensor_mul(rm[:], va[:], mu[:])
    rstd_bf = pool.tile([1, TOK], BF16, name="ln_rstd_bf")
    V.tensor_copy(rstd_bf[:], va[:])
    rm_bf = pool.tile([1, TOK], BF16, name="ln_rm_bf")
    V.tensor_copy(rm_bf[:], rm[:])

    for ci in range(8):
        for nt in range(2):
            a_ps = psum.tile([P, 512], F32, name="lnA_ps")
            b_ps = psum.tile([P, 512], F32, name="lnB_ps")
            tsl = slice(nt * 512, (nt + 1) * 512)
            mm(a_ps[:], sp_row[0:1, ci * P:(ci + 1) * P], rstd_bf[0:1, tsl],
               start=True, stop=True)
            mm(b_ps[:], sp_row[0:1, ci * P:(ci + 1) * P], rm_bf[0:1, tsl],
               start=True, stop=True)
            sl = slice(ci * TOK + nt * 512, ci * TOK + (nt + 1) * 512)
            t1 = tmp.tile([P, 512], BF16, name="ln_t1")
            V.tensor_mul(t1[:], xc[:, sl], a_ps[:])
            V.scalar_tensor_tensor(out_bf[:, sl], t1[:],
                                   sh_col[:, ci:ci + 1], b_ps[:],
                                   ALU.add, ALU.subtract)
        if block_done is not None and ci % 2 == 1:
            block_done(ci // 2)


def qkv_phase(nc, tc, io, dbg, slab, tmp, rhsp,
              h1_all, a2a_in, mm, mm1,
              Cb, Sb, sel2, sel2T, ones128, ones65, qb_col, vb128, perm_sb,
              ident_sb, scale_sb):
    V = nc.vector
    S = nc.scalar
    dma = nc.sync.dma_start

    w_sb = slab.tile([P, 8 * 384], BF16, name="w_sb")
    dma(out=w_sb[:].rearrange("p (blk c) -> p blk c", blk=8),
        in_=io["w_qkv"].ap().rearrange("(blk p) c -> p blk c", p=P))

    qn = slab.tile([P, B * L], BF16, name="qn")      # (128, 8192)
    kn = slab.tile([P, B * L], BF16, name="kn")
    v_sb = slab.tile([P, B * 16 * 2 * 65], BF16, name="v_sb")
    V.memset(v_sb[:].rearrange("p (blk c) -> p blk c", c=65)[:, :, 64:65], 1.0)
    invk_raw = slab.tile([P, P], F32, name="invk_raw")

    qkv_loop(nc, tc, io, slab, tmp, rhsp, h1_all, mm, mm1,
             Cb, Sb, sel2, sel2T, ones128, qb_col, perm_sb, ident_sb,
             scale_sb, w_sb, qn, kn, v_sb, invk_raw)

    invk = slab.tile([P, P], F32, name="invk")
    S.activation(invk[:], invk_raw[:], AF.Sqrt)
    V.tensor_scalar_max(invk[:], invk[:], 1e-12)
    V.reciprocal_approx_fast(invk[:], invk[:])

    for name, t in (("qn", qn), ("kn", kn), ("v_sb", v_sb)):
        if name in dbg:
            dma(out=dbg[name][:], in_=t[:])

    attention(nc, tc, dbg, slab, tmp, a2a_in, mm, mm1,
              ones65, vb128, qn, kn, v_sb, invk)


def qkv_loop(nc, tc, io, slab, tmp, rhsp, h1_all, mm, mm1,
             Cb, Sb, sel2, sel2T, ones128, qb_col, perm_sb, ident_sb,
             scale_sb, w_sb, qn, kn, v_sb, invk_raw):
    V = nc.vector
    S = nc.scalar
    dma = nc.sync.dma_start
    psum_cm = tc.tile_pool(name="qkvpsum", bufs=1, space="PSUM")
    psum = psum_cm.__enter__()

    def process_q(q_ps, blk, nt):
        gsl = slice(blk * TOK + nt * 512, blk * TOK + (nt + 1) * 512)
        cpos = (blk % 2) * TOK + nt * 512
        csl = slice(cpos, cpos + 512)
        qb = tmp.tile([P, 512], BF16, name="qb")
        V.tensor_scalar_add(qb[:], q_ps[:], qb_col[:, 0:1])
        q2 = tmp.tile([P, 512], BF16, name="q2")
        V.tensor_mul(q2[:], qb[:], qb[:])
        sq_ps = psum.tile([2, 512], F32, name="sq_ps", tag="red")
        mm1(sq_ps[:], sel2[:], q2[:])
        qsd = tmp.tile([2, 512], F32, name="qsd")
        S.activation(qsd[:], sq_ps[:], AF.Sqrt)
        V.tensor_scalar_max(qsd[:], qsd[:], 1e-12)
        iq = tmp.tile([2, 512], F32, name="iq")
        V.reciprocal_approx_fast(iq[:], qsd[:])
        iq_bf = tmp.tile([2, 512], BF16, name="iq_bf")
        V.tensor_scalar_mul(iq_bf[:], iq[:], scale_sb[:, 0:1])
        swp_ps = psum.tile([P, 512], F32, name="swp_ps", tag="bcast")
        mm1(swp_ps[:], perm_sb[:], qb[:])
        t1 = tmp.tile([P, 512], BF16, name="rope_t1")
        t2 = tmp.tile([P, 512], BF16, name="rope_t2")
        V.tensor_mul(t1[:], qb[:], Cb[:, csl])
        V.tensor_mul(t2[:], swp_ps[:], Sb[:, csl])
        qr = tmp.tile([P, 512], BF16, name="qr")
        V.tensor_add(qr[:], t1[:], t2[:])
        ib_ps = psum.tile([P, 512], F32, name="ib_ps", tag="bcast")
        mm1(ib_ps[:], sel2T[:], iq_bf[:])
        V.tensor_mul(qn[:, gsl], qr[:], ib_ps[:])

    def process_k(k_ps, blk, nt):
        gsl = slice(blk * TOK + nt * 512, blk * TOK + (nt + 1) * 512)
        cpos = (blk % 2) * TOK + nt * 512
        csl = slice(cpos, cpos + 512)
        b_idx = blk // 2
        kb = tmp.tile([P, 512], BF16, name="kb")
        V.tensor_copy(kb[:], k_ps[:])
        k2 = tmp.tile([P, 512], BF16, name="k2")
        V.tensor_mul(k2[:], kb[:], kb[:])
        ks_ps = psum.tile([P, 8], F32, name="ks_ps", tag="red")
        for hh in range(2):
            for t4 in range(4):
                mm1(ks_ps[:, hh * 4 + t4:hh * 4 + t4 + 1],
                    k2[hh * 64:(hh + 1) * 64, t4 * 128:(t4 + 1) * 128],
                    ones128[hh * 64:(hh + 1) * 64, 0:1])
        kt0 = (blk % 2) * 8 + nt * 4
        base = (b_idx * 16 + kt0) * 2
        V.tensor_copy(
            invk_raw[:, base:base + 8]
            .rearrange("p (t4 h) -> p h t4", h=2),
            ks_ps[:].rearrange("p (h t4) -> p h t4", h=2))
        kswp_ps = psum.tile([P, 512], F32, name="kswp_ps", tag="bcast")
        mm1(kswp_ps[:], perm_sb[:], kb[:])
        t1 = tmp.tile([P, 512], BF16, name="rope_t1")
        t2 = tmp.tile([P, 512], BF16, name="rope_t2")
        V.tensor_mul(t1[:], kb[:], Cb[:, csl])
        V.tensor_mul(t2[:], kswp_ps[:], Sb[:, csl])
        V.tensor_add(kn[:, gsl], t1[:], t2[:])

    for blk in range(8):
        b_idx = blk // 2
        rhs = {}
        for nt in range(2):
            for ci in range(8):
                r = rhsp.tile([P, 512], BF16, name="h1r")
                dma(out=r[:], in_=h1_all[0][blk, ci * P:(ci + 1) * P,
                                            nt * 512:(nt + 1) * 512])
                rhs[(nt, ci)] = r

        # q/k accumulation; lhsT reused across nt (one LDW per 2 matmuls)
        acc = {}
        for wname in ("q", "k"):
            for nt in range(2):
                acc[(wname, nt)] = psum.tile(
                    [P, 512], F32, name=f"{wname}{nt}_ps",
                    tag=f"{wname}{nt}_ps")
        for ci in range(8):
            for w_off, wname in ((0, "q"), (128, "k")):
                wt = w_sb[:, ci * 384 + w_off:ci * 384 + w_off + 128]
                for nt in range(2):
                    mm(acc[(wname, nt)][:], wt, rhs[(nt, ci)][:],
                       start=(ci == 0), stop=(ci == 7))
        for nt in range(2):
            process_q(acc[("q", nt)], blk, nt)
            process_k(acc[("k", nt)], blk, nt)

        # v: co-major matmul then PE transpose to token-major
        for nt in range(2):
            v_ps = psum.tile([P, 512], F32, name="v_ps", tag="vtp", bufs=2)
            for ci in range(8):
                mm(v_ps[:], w_sb[:, ci * 384 + 256:ci * 384 + 384],
                   rhs[(nt, ci)][:], start=(ci == 0), stop=(ci == 7))
            vco = tmp.tile([P, 512], BF16, name="vco")
            V.tensor_copy(vco[:], v_ps[:])
            kt0 = (blk % 2) * 8 + nt * 4
            for t4 in range(4):
                tp_ps = psum.tile([P, P], BF16, name="tp_ps", tag="vtp",
                                  bufs=2)
                nc.tensor.transpose(tp_ps[:], vco[:, t4 * 128:(t4 + 1) * 128],
                                    ident_sb[:])
                kt = kt0 + t4
                vbase = (b_idx * 16 + kt) * 2 * 65
                V.tensor_copy(
                    v_sb[:, vbase:vbase + 130]
                    .rearrange("p (h c) -> p h c", h=2)[:, :, 0:64],
                    tp_ps[:].rearrange("p (h c) -> p h c", h=2))
    psum_cm.__exit__(None, None, None)


def attention(nc, tc, dbg, slab, tmp, a2a_in, mm, mm1,
              ones65, vb128, qn, kn, v_sb, invk):
    V = nc.vector
    S = nc.scalar
    dma = nc.sync.dma_start
    psum_cm = tc.tile_pool(name="atnpsum", bufs=1, space="PSUM")
    psum = psum_cm.__enter__()
    attn = slab.tile([P, B * L], BF16, name="attn")

    pending = []

    # eviction: per (b, qh, hh, j) the pv (65, 512) -> attn rows hh*64..
    def evict(b2, q2, items):
        for (hh, j, pvall) in items:
            rec = tmp.tile([65, 512], F32, name="rec")
            V.reciprocal_approx_fast(rec[:], pvall[:])
            rec_bf = tmp.tile([65, 512], BF16, name="rec_bf")
            V.tensor_copy(rec_bf[64:65, :], rec[64:65, :])
            rb_ps = psum.tile([P, 1024], F32, name="rb_ps",
                              tag="s_h0")
            mm(rb_ps[hh * 64:hh * 64 + 64, 0:512], ones65[64:65, :],
               rec_bf[64:65, :], start=True, stop=True)
            tm = tmp.tile([P, 512], BF16, name="tm")
            V.tensor_mul(tm[hh * 64:(hh + 1) * 64, :], pvall[0:64, :],
                         rb_ps[hh * 64:hh * 64 + 64, 0:512])
            col = b2 * L + q2 * 1024 + j * 512
            V.tensor_scalar_add(attn[hh * 64:(hh + 1) * 64, col:col + 512],
                                tm[hh * 64:(hh + 1) * 64, :],
                                vb128[hh * 64:(hh + 1) * 64, 0:1])

    for b_idx in range(B):
        for qh in range(2):
            pv = {}
            for hh in range(2):
                for j in range(2):
                    pv[(hh, j)] = psum.tile(
                        [65, 512], F32, name=f"pv{hh}{j}", tag=f"pv{hh}{j}")

            def drain(item):
                ktd, es = item
                for hh in range(2):
                    vb = ((b_idx * 16 + ktd) * 2 + hh) * 65
                    for j in range(2):
                        mm(pv[(hh, j)][:], v_sb[:, vb:vb + 65],
                           es[hh][:, j * 512:(j + 1) * 512],
                           start=(ktd == 0), stop=(ktd == 15))

            pend = []
            for kt in range(16):
                ksl = slice(b_idx * L + kt * 128, b_idx * L + (kt + 1) * 128)
                sh = []
                for hh in range(2):
                    s_h = psum.tile([P, 1024], F32, name=f"s_h{hh}",
                                    tag=f"s_h{hh}")
                    sh.append(s_h)
                # interleave heads so row-groups 0-63 / 64-127 overlap in PE
                for j in range(2):
                    qsl = slice(b_idx * L + qh * 1024 + j * 512,
                                b_idx * L + qh * 1024 + (j + 1) * 512)
                    for hh in range(2):
                        hs = slice(hh * 64, (hh + 1) * 64)
                        mm1(sh[hh][:, j * 512:(j + 1) * 512],
                            kn[hs, ksl], qn[hs, qsl])
                es = []
                for hh in range(2):
                    e_bf = tmp.tile([P, 1024], BF16, name="e_bf", bufs=6)
                    ikcol = (b_idx * 16 + kt) * 2 + hh
                    S.activation(e_bf[:], sh[hh][:], AF.Exp,
                                 scale=invk[:, ikcol:ikcol + 1])
                    es.append(e_bf)
                pend.append((kt, es))
                if len(pend) > 2:
                    drain(pend.pop(0))
                if kt in (6, 8, 10, 12) and pending:
                    b2, q2, items = pending[0]
                    evict(b2, q2, [items.pop(0)])
                    if not items:
                        pending.pop(0)
            for item in pend:
                drain(item)

            items = []
            for hh in range(2):
                for j in range(2):
                    pvall = tmp.tile([65, 512], F32, name="pvall", bufs=8)
                    V.tensor_copy(pvall[:], pv[(hh, j)][:])
                    items.append((hh, j, pvall))
            pending.append((b_idx, qh, items))
    while pending:
        b2, q2, items = pending.pop(0)
        evict(b2, q2, items)

    psum_cm.__exit__(None, None, None)
    if "attn" in dbg:
        dma(out=dbg["attn"][:], in_=attn[:])
    dma(out=a2a_in[:].rearrange("blk p t -> p blk t"),
        in_=attn[:].rearrange("p (blk t) -> p blk t", blk=8))


# ---------------------------------------------------------------------------
# host-side input preparation
# ---------------------------------------------------------------------------

_PERM = np.concatenate([np.arange(0, HD, 2), np.arange(1, HD, 2)])  # re|im


def _perm_matrix():
    """(128,128) with entry (swap(m), m) = 1; swap exchanges the re (0:32)
    and im (32:64) halves of each 64-row head slice."""
    pm = np.zeros((P, P), np.float32)
    for m in range(P):
        base = (m // 64) * 64
        r = m - base
        sw = base + (r + 32) % 64
        pm[sw, m] = 1.0
    return pm


def prep_in_maps(inputs):
    import ml_dtypes
    bf = lambda a: np.ascontiguousarray(a).astype(ml_dtypes.bfloat16)
    f32 = lambda a: np.ascontiguousarray(np.asarray(a, dtype=np.float32))

    x = np.asarray(inputs["x"], np.float32)
    cond = np.asarray(inputs["cond_BD"], np.float32)
    W_qkv = np.asarray(inputs["W_qkv"], np.float32)
    q_bias = np.asarray(inputs["q_bias"], np.float32)
    v_bias = np.asarray(inputs["v_bias"], np.float32)
    sml = np.asarray(inputs["scale_mul_log"], np.float32).reshape(H)
    W_proj = np.asarray(inputs["W_proj"], np.float32)
    b_proj = np.asarray(inputs["b_proj"], np.float32)
    W_fc1 = np.asarray(inputs["W_fc1"], np.float32)
    b_fc1 = np.asarray(inputs["b_fc1"], np.float32)
    W_fc2 = np.asarray(inputs["W_fc2"], np.float32)
    b_fc2 = np.asarray(inputs["b_fc2"], np.float32)
    W_ada = np.asarray(inputs["W_ada"], np.float32)
    b_ada = np.asarray(inputs["b_ada"], np.float32)
    fc = np.asarray(inputs["freqs_cos"], np.float32)
    fs = np.asarray(inputs["freqs_sin"], np.float32)

    cosT = f32(fc.T)   # (32, L)
    sinT = f32(fs.T)
    w_projT = bf(W_proj.T)
    w_fc1T = bf(W_fc1.T)
    w_fc2T = bf(W_fc2.T)
    b_proj_c = f32(b_proj.reshape(8, P).T)
    b_fc1_c = f32(b_fc1.reshape(32, P).T)
    b_fc2_c = f32(b_fc2.reshape(8, P).T)
    w_adaT = W_ada.T  # (1024, 6144)

    in_maps = []
    for m in range(N_CORES):
        b_own, pm = m // 2, m % 2
        h0, h1 = 2 * m, 2 * m + 1
        cols = []
        for h in (h0, h1):
            cols.append(W_qkv[h * HD + _PERM, :])          # q rows, permuted
        for h in (h0, h1):
            cols.append(W_qkv[C + h * HD + _PERM, :])      # k rows, permuted
        for h in (h0, h1):
            cols.append(W_qkv[2 * C + h * HD:2 * C + (h + 1) * HD, :])  # v
        w_qkv_m = bf(np.concatenate(cols, axis=0).T)       # (1024, 384)
        qkv_b_m = np.concatenate([
            q_bias[h0 * HD + _PERM], q_bias[h1 * HD + _PERM],
            np.zeros(P, np.float32),
            v_bias[h0 * HD:(h0 + 1) * HD], v_bias[h1 * HD:(h1 + 1) * HD],
        ]).reshape(384, 1)

        vb2 = np.stack([v_bias[h0 * HD:(h0 + 1) * HD],
                        v_bias[h1 * HD:(h1 + 1) * HD]], axis=1)
        vb128 = np.concatenate([v_bias[h0 * HD:(h0 + 1) * HD],
                                v_bias[h1 * HD:(h1 + 1) * HD]]).reshape(P, 1)
        xm = x[b_own, pm * TOK:(pm + 1) * TOK, :]          # (1024, 1024)
        identm = np.eye(P, dtype=np.float32)
        s2t = np.zeros((2, P), np.float32)
        s2t[0, 0:64] = 1.0
        s2t[1, 64:128] = 1.0
        in_maps.append({
            "perm_m": bf(_perm_matrix()),
            "sel2t": bf(s2t),
            "ident": bf(identm),
            "vb2": f32(vb2),
            "vb128": f32(vb128),
            "xT": f32(xm.T),
            "condT": f32(cond[b_own].reshape(C, 1)),
            "w_ada": bf(w_adaT[:, pm * 3 * C:(pm + 1) * 3 * C]),
            "b_ada_r": f32(b_ada[pm * 3 * C:(pm + 1) * 3 * C].reshape(1, -1)),
            "w_qkv": w_qkv_m,
            "qkv_b": f32(qkv_b_m),
            "scale_log": f32(sml[[h0, h1]].reshape(2, 1)),
            "cosT": cosT, "sinT": sinT,
            "w_proj": w_projT, "b_proj_c": b_proj_c,
            "w_fc1": w_fc1T, "b_fc1_c": b_fc1_c,
            "w_fc2": w_fc2T, "b_fc2_c": b_fc2_c,
        })
    return in_maps


_NC_CACHE = {}


def _get_nc(debug_outputs=()):
    key = tuple(sorted(debug_outputs))
    if key not in _NC_CACHE:
        _NC_CACHE[key] = build_nc(debug_outputs)
    return _NC_CACHE[key]


def run(inputs, debug_outputs=(), trace=False):
    nc = _get_nc(debug_outputs)
    in_maps = prep_in_maps(inputs)
    res = run_bass_kernel_spmd(nc, in_maps, core_ids=list(range(N_CORES)),
                               trace=trace)
    out = np.empty((B, L, C), np.float32)
    for m in range(N_CORES):
        b_own, pm = m // 2, m % 2
        out[b_own, pm * TOK:(pm + 1) * TOK, :] = res.results[m]["outT"].T
    return out, res


def kernel(**inputs):
    out, _ = run(inputs)
    return out


# revision 34
# speedup vs baseline: 1.1299x; 1.0126x over previous
"""Trainium2 Bass kernel for nn_AdaLNSelfAttn_RoPE (B=4, L=2048, C=1024, H=16).

Sharding across 8 NeuronCores (one chip):
  - Tokens flattened (B*L = 8192) and sharded 1024/core; each core owns half of
    one batch.  LayerNorms, AdaLN modulation, proj, and the FFN run on the
    token shard (sequence parallel).
  - QKV + attention are head-sharded: core m owns heads {2m, 2m+1}.  The
    modulated hidden h1 is AllGather'ed (bf16) so every core computes QKV for
    its two heads over all tokens.  Attention outputs return to token shards
    via AllToAll.
  - AdaLN (silu+linear) is column-sharded across pairs of cores (both cores of
    a pair own the same batch) with a 2-wide AllGather.
  - Activations are channel-major (C on partitions) so matmuls chain without
    transposes; per-token statistics (LN mean/rstd, softmax rowsums, q-norms)
    are reduced across partitions with ones-matmuls and broadcast back with
    outer-product matmuls.
  - Matmuls in bf16 (fp32 PSUM accumulation); statistics in fp32.
  - Softmax skips max-subtraction (cosine attention bounds scores to [-4,4]).
    The softmax denominator rides as a 65th output column of the PV matmul;
    the k-side 1/||k|| rides as the per-partition scale of the exp()
    activation.
  - RoPE pairs are pre-permuted on the host (re parts in rows 0-31 of each
    head slice, im parts in rows 32-63) so rotation is four 32-row block ops.
"""

import numpy as np

import concourse.bass as bass
import concourse.bacc as bacc
import concourse.mybir as mybir
import concourse.tile as tile
from concourse.bass_utils import run_bass_kernel_spmd

F32 = mybir.dt.float32
BF16 = mybir.dt.bfloat16
FP8 = mybir.dt.float8e4
W8SCALE = 64.0
AF = mybir.ActivationFunctionType
ALU = mybir.AluOpType

B, L, C, H = 4, 2048, 1024, 16
HD = C // H          # 64
N_CORES = 8
TOK = (B * L) // N_CORES   # 1024 tokens per core
MAX_SCALE = float(np.log(100.0))
LN_EPS = 1e-6
P = 128


def build_nc(debug_outputs=()):
    nc = bacc.Bacc("TRN2", target_bir_lowering=False, debug=False,
                   num_devices=N_CORES)

    dt = nc.dram_tensor
    io = {}
    io["xT"] = dt("xT", [C, TOK], F32, kind="ExternalInput")
    io["condT"] = dt("condT", [C, 1], F32, kind="ExternalInput")
    io["w_ada"] = dt("w_ada", [C, 3 * C], BF16, kind="ExternalInput")
    io["b_ada_r"] = dt("b_ada_r", [1, 3 * C], F32, kind="ExternalInput")
    io["w_qkv"] = dt("w_qkv", [C, 384], BF16, kind="ExternalInput")
    io["qkv_b"] = dt("qkv_b", [384, 1], F32, kind="ExternalInput")
    io["perm_m"] = dt("perm_m", [P, P], BF16, kind="ExternalInput")
    io["sel2t"] = dt("sel2t", [2, P], BF16, kind="ExternalInput")
    io["ident"] = dt("ident", [P, P], BF16, kind="ExternalInput")
    io["vb2"] = dt("vb2", [64, 2], F32, kind="ExternalInput")
    io["vb128"] = dt("vb128", [P, 1], F32, kind="ExternalInput")
    io["scale_log"] = dt("scale_log", [2, 1], F32, kind="ExternalInput")
    io["cosT"] = dt("cosT", [32, L], F32, kind="ExternalInput")
    io["sinT"] = dt("sinT", [32, L], F32, kind="ExternalInput")
    io["w_proj"] = dt("w_proj", [C, C], BF16, kind="ExternalInput")
    io["b_proj_c"] = dt("b_proj_c", [P, 8], F32, kind="ExternalInput")
    io["w_fc1"] = dt("w_fc1", [C, 4 * C], FP8, kind="ExternalInput")
    io["b_fc1_c"] = dt("b_fc1_c", [P, 32], F32, kind="ExternalInput")
    io["w_fc2"] = dt("w_fc2", [4 * C, C], FP8, kind="ExternalInput")
    io["b_fc2_c"] = dt("b_fc2_c", [P, 8], F32, kind="ExternalInput")
    io["outT"] = dt("outT", [C, TOK], F32, kind="ExternalOutput")

    dbg = {}
    for name, shape, dtp in [
        ("h1_all", [8, C, TOK], BF16),
        ("qn", [P, B * L], BF16),
        ("kn", [P, B * L], BF16),
        ("v_sb", [P, B * 16 * 2 * 65], BF16),
        ("attn", [P, B * L], BF16),
        ("a2a_out", [8, P, TOK], BF16),
        ("x2", [P, 8 * TOK], F32),
        ("ada_all", [2, 3 * C], F32),
    ]:
        if name in debug_outputs:
            dbg[name] = dt("dbg_" + name, shape, dtp, kind="ExternalOutput")

    with tile.TileContext(nc) as tc:
        _body(nc, tc, io, dbg)
    nc.compile()
    return nc


def _body(nc, tc, io, dbg):
    mm = nc.tensor.matmul
    V = nc.vector
    S = nc.scalar
    dma = nc.sync.dma_start

    def mm1(out, lhsT, rhs):
        mm(out, lhsT, rhs, start=True, stop=True)

    with tc.tile_pool(name="dram", bufs=1, space="DRAM") as dram, \
         tc.tile_pool(name="const", bufs=1) as const, \
         tc.tile_pool(name="outer", bufs=1) as outer:

        # DRAM bounce buffers for collectives
        ada_in = dram.tile([1, 3 * C], F32, name="ada_in")
        ada_all = dram.tile([2, 3 * C], F32, name="ada_all")
        h1_in = dram.tile([TOK, C], BF16, name="h1_in")
        h1_all = [dram.tile([8, C, TOK], BF16, addr_space="Shared",
                            name="h1_all0")]
        a2a_in = dram.tile([8, P, TOK], BF16, name="a2a_in")
        a2a_out = dram.tile([8, P, TOK], BF16, name="a2a_out")

        # ------------------------------------------------------- constants
        ones128 = const.tile([P, 1], BF16, name="ones128")
        V.memset(ones128[:], 1.0)
        ones64c = const.tile([64, 1], BF16, name="ones64c")
        V.memset(ones64c[:], 1.0)
        ones1x64 = const.tile([1, 64], BF16, name="ones1x64")
        V.memset(ones1x64[:], 1.0)
        sel2 = const.tile([P, 2], BF16, name="sel2")     # head indicator cols
        V.memset(sel2[:], 0.0)
        V.memset(sel2[0:64, 0:1], 1.0)
        V.memset(sel2[64:128, 1:2], 1.0)
        sel2T = const.tile([2, P], BF16, name="sel2T")   # head indicator rows
        dma(out=sel2T[:], in_=io["sel2t"][:, :])

        # rope cos (128, 2048) bf16: 4 vertical copies of (32, 2048); and
        # sign-baked sin: rows [0:32]=-sin [32:64]=+sin [64:96]=-sin [96:]=+sin
        Cb = const.tile([P, L], BF16, name="Cb")
        Sb = const.tile([P, L], BF16, name="Sb")
        with tc.tile_pool(name="cs_pool", bufs=1) as csp:
            cs_f = csp.tile([P, L], F32, name="cs_f")
            for j in range(4):
                dma(out=cs_f[32 * j:32 * (j + 1), :], in_=io["cosT"][:, :])
            V.tensor_copy(Cb[:], cs_f[:])
            sn_f = csp.tile([P, L], F32, name="sn_f")
            for j in range(4):
                dma(out=sn_f[32 * j:32 * (j + 1), :], in_=io["sinT"][:, :])
            for j in range(4):
                sgn = -1.0 if j % 2 == 0 else 1.0
                S.activation(Sb[32 * j:32 * (j + 1), :],
                             sn_f[32 * j:32 * (j + 1), :], AF.Copy, scale=sgn)
        perm_sb = const.tile([P, P], BF16, name="perm_sb")
        dma(out=perm_sb[:], in_=io["perm_m"][:, :])
        ident_sb = const.tile([P, P], BF16, name="ident_sb")
        dma(out=ident_sb[:], in_=io["ident"][:, :])
        vb2 = const.tile([64, 2], F32, name="vb2")
        dma(out=vb2[:], in_=io["vb2"][:, :])
        vb128 = const.tile([P, 1], F32, name="vb128")
        dma(out=vb128[:], in_=io["vb128"][:, :])
        ones65 = const.tile([65, 64], BF16, name="ones65")
        V.memset(ones65[:], 1.0)

        # bias columns
        qb_col = const.tile([P, 3], F32, name="qb_col")
        dma(out=qb_col[:], in_=io["qkv_b"].ap().rearrange("(m p) 1 -> p m", p=P))
        bproj = const.tile([P, 8], F32, name="bproj")
        dma(out=bproj[:], in_=io["b_proj_c"][:, :])
        bfc1 = const.tile([P, 32], F32, name="bfc1")
        dma(out=bfc1[:], in_=io["b_fc1_c"][:, :])
        bfc2 = const.tile([P, 8], F32, name="bfc2")
        dma(out=bfc2[:], in_=io["b_fc2_c"][:, :])
        bada = const.tile([1, 3 * C], F32, name="bada")
        dma(out=bada[:], in_=io["b_ada_r"][:, :])
        epsc = const.tile([1, 1], F32, name="epsc")
        V.memset(epsc[:], LN_EPS)

        # scale_mul = exp(min(scale_log, MAX_SCALE))
        scale_sb = const.tile([2, 1], F32, name="scale_sb")
        with tc.tile_pool(name="scp", bufs=1) as scp:
            sc_raw = scp.tile([2, 1], F32, name="sc_raw")
            dma(out=sc_raw[:], in_=io["scale_log"][:, :])
            sc_min = scp.tile([2, 1], F32, name="sc_min")
            V.tensor_scalar_min(sc_min[:], sc_raw[:], MAX_SCALE)
            S.activation(scale_sb[:], sc_min[:], AF.Exp)

        # --------------------------------------------------- phase 0: adaLN
        ada_phase(nc, tc, io, dram, const, ada_in, ada_all)

        g1c = ada_col(nc, const, ada_all, 0, "g1c")
        g2c = ada_col(nc, const, ada_all, 1, "g2c")
        sh1c = ada_col(nc, const, ada_all, 4, "sh1c")
        sh2c = ada_col(nc, const, ada_all, 5, "sh2c")
        s1p = ada_srow(nc, tc, const, ada_all, 2, "s1p")
        s2p = ada_srow(nc, tc, const, ada_all, 3, "s2p")

        if "ada_all" in dbg:
            dma(out=dbg["ada_all"][:], in_=ada_all[:])

        # ------------------------------------------------------ LN1 -> AG h1
        with tc.tile_pool(name="ln1pool", bufs=1) as lp, \
             tc.tile_pool(name="ln1psum", bufs=2, space="PSUM") as lps, \
             tc.tile_pool(name="ln1tmp", bufs=3) as ltmp:
            xT_sb = lp.tile([P, 8 * TOK], F32, name="xT_sb")
            dma(out=xT_sb[:].rearrange("p (blk t) -> p blk t", blk=8),
                in_=io["xT"].ap().rearrange("(blk p) t -> p blk t", p=P))
            h1_bf = lp.tile([P, 8 * TOK], BF16, name="h1_bf")

            def h1_chunk_out(ci2):
                # stage channel blocks as they complete; one AG at the end
                csl = slice(2 * ci2 * P, 2 * (ci2 + 1) * P)
                dma(out=h1_in[csl, :].rearrange("(blk p) t -> p blk t", p=P),
                    in_=h1_bf[:, 2 * ci2 * TOK:2 * (ci2 + 1) * TOK]
                    .rearrange("p (blk t) -> p blk t", blk=2))

            layer_norm(nc, lp, lps, ltmp, xT_sb, s1p, sh1c, h1_bf,
                       ones128, epsc, mm, block_done=h1_chunk_out)
        nc.gpsimd.collective_compute(
            "AllGather", ALU.bypass,
            replica_groups=[list(range(N_CORES))],
            ins=[h1_in[:].opt()], outs=[h1_all[0][:].opt()])
        if "h1_all" in dbg:
            dma(out=dbg["h1_all"][:], in_=h1_all[0][:])

        # ------------------------------------- phase 2+3: qkv, rope, attention
        with tc.tile_pool(name="atslab", bufs=1) as ats, \
             tc.tile_pool(name="attmp", bufs=2) as atmp, \
             tc.tile_pool(name="atrhs", bufs=18) as arhs:
            qkv_phase(nc, tc, io, dbg, ats, atmp, arhs,
                      h1_all, a2a_in, mm, mm1,
                      Cb, Sb, sel2, sel2T, ones128, ones65,
                      qb_col, vb128, perm_sb, ident_sb, scale_sb)
        nc.gpsimd.collective_compute(
            "AllToAll", ALU.bypass,
            replica_groups=[list(range(N_CORES))],
            ins=[a2a_in[:].opt()], outs=[a2a_out[:].opt()])
        if "a2a_out" in dbg:
            dma(out=dbg["a2a_out"][:], in_=a2a_out[:])

        # ----------------------------------------- phase 4: proj + residual
        x2p_cm = tc.tile_pool(name="x2pool", bufs=1)
        x2p = x2p_cm.__enter__()
        x2_sb = x2p.tile([P, 8 * TOK], F32, name="x2_sb")
        with tc.tile_pool(name="pjpool", bufs=1) as pjp, \
             tc.tile_pool(name="pjpsum", bufs=2, space="PSUM") as pjps, \
             tc.tile_pool(name="pjtmp", bufs=3) as ptmp:
            wp_sb = pjp.tile([P, 8 * C], BF16, name="wp_sb")
            dma(out=wp_sb[:].rearrange("p (blk c) -> p blk c", blk=8),
                in_=io["w_proj"].ap().rearrange("(blk p) c -> p blk c", p=P))
            prhs = []
            for r in range(8):
                for nt in range(2):
                    t = pjp.tile([P, 512], BF16, name=f"prhs_{r}_{nt}",
                                 tag="prhs", bufs=16)
                    dma(out=t[:], in_=a2a_out[r, :, nt * 512:(nt + 1) * 512])
                    prhs.append(t)
            for cot in range(8):
                xres = ptmp.tile([P, 2 * 512], F32, name="xres")
                dma(out=xres[:],
                    in_=io["xT"][cot * P:(cot + 1) * P, :])
                pp = [pjps.tile([P, 512], F32, name=f"p_ps{nt}",
                                tag=f"p_ps{nt}") for nt in range(2)]
                for r in range(8):
                    wt = wp_sb[:, r * C + cot * P:r * C + (cot + 1) * P]
                    for nt in range(2):
                        mm(pp[nt][:], wt, prhs[r * 2 + nt][:],
                           start=(r == 0), stop=(r == 7))
                for nt in range(2):
                    t1 = ptmp.tile([P, 512], F32, name="pj_t1")
                    V.tensor_scalar(t1[:], pp[nt][:], bproj[:, cot:cot + 1],
                                    g1c[:, cot:cot + 1], ALU.add, ALU.mult)
                    sl = slice(cot * TOK + nt * 512, cot * TOK + (nt + 1) * 512)
                    V.tensor_add(x2_sb[:, sl], t1[:],
                                 xres[:, nt * 512:(nt + 1) * 512])
        if "x2" in dbg:
            dma(out=dbg["x2"][:], in_=x2_sb[:])

        # ------------------------------------------------- phase 5-7: LN2+FFN
        with tc.tile_pool(name="ffnpool", bufs=1) as fp, \
             tc.tile_pool(name="ffntmp", bufs=3) as ftmp:
            h2_bf = fp.tile([P, 8 * TOK], FP8, name="h2_bf")
            with tc.tile_pool(name="ln2pool", bufs=1) as lp2, \
                 tc.tile_pool(name="ln2psum", bufs=2, space="PSUM") as lps2:
                layer_norm(nc, lp2, lps2, ftmp, x2_sb, s2p, sh2c, h2_bf,
                           ones128, epsc, mm)

            g2s = ftmp.tile([P, 8], F32, name="g2s", bufs=1)
            V.tensor_scalar_mul(g2s[:], g2c[:], 1.0 / W8SCALE)
            fw_cm = tc.tile_pool(name="ffnw", bufs=2)
            fw = fw_cm.__enter__()
            fps_cm = tc.tile_pool(name="ffnpsum", bufs=4, space="PSUM")
            fps = fps_cm.__enter__()
            hact = fp.tile([P, 32 * TOK], FP8, name="hact")
            h2v = h2_bf[:].rearrange("p (blk t) -> p blk t", blk=8)
            for cot in range(32):
                w1 = fw.tile([P, 8 * P], FP8, name="w1")
                dma(out=w1[:].rearrange("p (blk c) -> p blk c", blk=8),
                    in_=io["w_fc1"].ap()[:, cot * P:(cot + 1) * P]
                    .rearrange("(blk p) c -> p blk c", p=P))
                w1v = w1[:].rearrange("p (ki j c) -> p ki j c", ki=4, j=2)
                fpp = [fps.tile([P, 512], F32, name=f"f_ps{nt}",
                                tag=f"f_ps{nt}", bufs=2) for nt in range(2)]
                for ki in range(4):
                    wt = w1v[:, ki, :, :]
                    for nt in range(2):
                        rhs3 = h2v[:, 2 * ki:2 * ki + 2,
                                   nt * 512:(nt + 1) * 512]
                        mm(fpp[nt][:], wt, rhs3,
                           perf_mode=mybir.MatmulPerfMode.DoubleRow,
                           start=(ki == 0), stop=(ki == 3))
                for nt in range(2):
                    S.activation(
                        hact[:, cot * TOK + nt * 512:cot * TOK + (nt + 1) * 512],
                        fpp[nt][:], AF.Gelu_apprx_tanh,
                        bias=bfc1[:, cot:cot + 1], scale=1.0 / W8SCALE)

            hactv = hact[:].rearrange("p (blk t) -> p blk t", blk=32)
            for cot in range(8):
                w2 = fw.tile([P, 32 * P], FP8, name="w2")
                dma(out=w2[:].rearrange("p (blk c) -> p blk c", blk=32),
                    in_=io["w_fc2"].ap()[:, cot * P:(cot + 1) * P]
                    .rearrange("(blk p) c -> p blk c", p=P))
                w2v = w2[:].rearrange("p (ki j c) -> p ki j c", ki=16, j=2)
                opp = [fps.tile([P, 512], F32, name=f"o_ps{nt}",
                                tag=f"o_ps{nt}", bufs=2) for nt in range(2)]
                for ki in range(16):
                    wt = w2v[:, ki, :, :]
                    for nt in range(2):
                        rhs3 = hactv[:, 2 * ki:2 * ki + 2,
                                     nt * 512:(nt + 1) * 512]
                        mm(opp[nt][:], wt, rhs3,
                           perf_mode=mybir.MatmulPerfMode.DoubleRow,
                           start=(ki == 0), stop=(ki == 15))
                for nt in range(2):
                    t1 = ftmp.tile([P, 512], F32, name="o_t1")
                    V.tensor_scalar(t1[:], opp[nt][:], bfc2[:, cot:cot + 1],
                                    g2s[:, cot:cot + 1], ALU.add, ALU.mult)
                    ot = ftmp.tile([P, 512], F32, name="ot")
                    sl = slice(cot * TOK + nt * 512, cot * TOK + (nt + 1) * 512)
                    V.tensor_add(ot[:], t1[:], x2_sb[:, sl])
                    dma(out=io["outT"][cot * P:(cot + 1) * P,
                                       nt * 512:(nt + 1) * 512],
                        in_=ot[:])
            fps_cm.__exit__(None, None, None)
            fw_cm.__exit__(None, None, None)
        x2p_cm.__exit__(None, None, None)


def ada_phase(nc, tc, io, dram, const, ada_in, ada_all):
    """silu(cond) @ W_ada_slice.T + b_ada, pair-wise AllGather."""
    mm = nc.tensor.matmul
    V = nc.vector
    S = nc.scalar
    dma = nc.sync.dma_start
    cond_sb = const.tile([P, 8], F32, name="cond_sb")
    dma(out=cond_sb[:],
        in_=io["condT"].ap().rearrange("(blk p) 1 -> p blk", p=P))
    scond = const.tile([P, 8], BF16, name="scond")
    S.activation(scond[:], cond_sb[:], AF.Silu)
    bada = const.tile([1, 3 * C], F32, name="bada2")
    dma(out=bada[:], in_=io["b_ada_r"][:, :])

    with tc.tile_pool(name="adapool", bufs=1) as ap_, \
         tc.tile_pool(name="adapsum", bufs=2, space="PSUM") as aps, \
         tc.tile_pool(name="adarhs", bufs=4) as arp:
        ada_sb = ap_.tile([1, 3 * C], F32, name="ada_sb")
        for nt6 in range(6):
            a_ps = aps.tile([1, 512], F32, name="a_ps")
            for ci in range(8):
                wt = arp.tile([P, 512], BF16, name="wt_ada")
                dma(out=wt[:], in_=io["w_ada"][ci * P:(ci + 1) * P,
                                               nt6 * 512:(nt6 + 1) * 512])
                mm(a_ps[:], scond[:, ci:ci + 1], wt[:],
                   start=(ci == 0), stop=(ci == 7))
            V.tensor_add(ada_sb[0:1, nt6 * 512:(nt6 + 1) * 512], a_ps[:],
                         bada[0:1, nt6 * 512:(nt6 + 1) * 512])
        dma(out=ada_in[:], in_=ada_sb[:])
    nc.gpsimd.collective_compute(
        "AllGather", ALU.bypass,
        replica_groups=[[0, 1], [2, 3], [4, 5], [6, 7]],
        ins=[ada_in[:].opt()], outs=[ada_all[:].opt()])


def ada_col(nc, const, ada_all, vec, name):
    r, off = (vec * C) // (3 * C), (vec * C) % (3 * C)
    t = const.tile([P, 8], F32, name=name)
    nc.sync.dma_start(out=t[:], in_=ada_all[r:r + 1, off:off + C]
                      .rearrange("1 (blk p) -> p blk", p=P))
    return t


def ada_srow(nc, tc, const, ada_all, vec, name):
    """(1, 1024) bf16 row of (ada_vec + 1)."""
    r, off = (vec * C) // (3 * C), (vec * C) % (3 * C)
    t = const.tile([1, C], BF16, name=name)
    with tc.tile_pool(name=name + "_f", bufs=1) as p:
        raw = p.tile([1, C], F32, name=name + "_raw")
        nc.sync.dma_start(out=raw[:], in_=ada_all[r:r + 1, off:off + C])
        nc.vector.tensor_scalar_add(t[:], raw[:], 1.0)
    return t


def layer_norm(nc, pool, psum, tmp, x_sb, sp_row, sh_col, out_bf, ones128, epsc, mm,
               block_done=None):
    """x_sb (128, 8192) f32, channel-major blocks; out_bf same layout bf16:
    LN(x) * (s+1) + sh, statistics over the channel (partition x block) dim."""
    V = nc.vector
    S = nc.scalar
    xc = pool.tile([P, 8 * TOK], BF16, name="ln_xc")
    for ci in range(8):
        sl = slice(ci * TOK, (ci + 1) * TOK)
        V.tensor_copy(xc[:, sl], x_sb[:, sl])
    mu_ps = [psum.tile([1, 512], F32, name=f"mu_ps{nt}", tag=f"mu_ps{nt}",
                       bufs=1) for nt in range(2)]
    s2_ps = [psum.tile([1, 512], F32, name=f"s2_ps{nt}", tag=f"s2_ps{nt}",
                       bufs=1) for nt in range(2)]
    for ci in range(8):
        sl = slice(ci * TOK, (ci + 1) * TOK)
        xsq = tmp.tile([P, TOK], BF16, name="ln_xsq")
        V.tensor_mul(xsq[:], xc[:, sl], xc[:, sl])
        for nt in range(2):
            tsl = slice(ci * TOK + nt * 512, ci * TOK + (nt + 1) * 512)
            mm(mu_ps[nt][:], ones128[:], xc[:, tsl],
               start=(ci == 0), stop=(ci == 7))
            mm(s2_ps[nt][:], ones128[:], xsq[:, nt * 512:(nt + 1) * 512],
               start=(ci == 0), stop=(ci == 7))
    mu = pool.tile([1, TOK], F32, name="ln_mu")
    va = pool.tile([1, TOK], F32, name="ln_va")
    for nt in range(2):
        tsl = slice(nt * 512, (nt + 1) * 512)
        S.activation(mu[0:1, tsl], mu_ps[nt][:], AF.Copy, scale=1.0 / C)
        S.activation(va[0:1, tsl], s2_ps[nt][:], AF.Copy, scale=1.0 / C)
    # va := rstd = 1/sqrt(va - mu^2 + eps), in place
    mu2 = pool.tile([1, TOK], F32, name="ln_mu2")
    V.tensor_mul(mu2[:], mu[:], mu[:])
    V.tensor_sub(va[:], va[:], mu2[:])
    S.activation(va[:], va[:], AF.Sqrt, bias=epsc[0:1, 0:1])
    V.reciprocal_approx_fast(va[:], va[:])
    rm = pool.tile([1, TOK], F32, name="ln_rm")
    V.tensor_mul(rm[:], va[:], mu[:])
    rstd_bf = pool.tile([1, TOK], BF16, name="ln_rstd_bf")
    V.tensor_copy(rstd_bf[:], va[:])
    rm_bf = pool.tile([1, TOK], BF16, name="ln_rm_bf")
    V.tensor_copy(rm_bf[:], rm[:])

    for ci in range(8):
        for nt in range(2):
            a_ps = psum.tile([P, 512], F32, name="lnA_ps")
            b_ps = psum.tile([P, 512], F32, name="lnB_ps")
            tsl = slice(nt * 512, (nt + 1) * 512)
            mm(a_ps[:], sp_row[0:1, ci * P:(ci + 1) * P], rstd_bf[0:1, tsl],
               start=True, stop=True)
            mm(b_ps[:], sp_row[0:1, ci * P:(ci + 1) * P], rm_bf[0:1, tsl],
               start=True, stop=True)
            sl = slice(ci * TOK + nt * 512, ci * TOK + (nt + 1) * 512)
            t1 = tmp.tile([P, 512], BF16, name="ln_t1")
            V.tensor_mul(t1[:], xc[:, sl], a_ps[:])
            V.scalar_tensor_tensor(out_bf[:, sl], t1[:],
                                   sh_col[:, ci:ci + 1], b_ps[:],
                                   ALU.add, ALU.subtract)
        if block_done is not None and ci % 2 == 1:
            block_done(ci // 2)


def qkv_phase(nc, tc, io, dbg, slab, tmp, rhsp,
              h1_all, a2a_in, mm, mm1,
              Cb, Sb, sel2, sel2T, ones128, ones65, qb_col, vb128, perm_sb,
              ident_sb, scale_sb):
    V = nc.vector
    S = nc.scalar
    dma = nc.sync.dma_start

    w_sb = slab.tile([P, 8 * 384], BF16, name="w_sb")
    dma(out=w_sb[:].rearrange("p (blk c) -> p blk c", blk=8),
        in_=io["w_qkv"].ap().rearrange("(blk p) c -> p blk c", p=P))

    qn = slab.tile([P, B * L], BF16, name="qn")      # (128, 8192)
    kn = slab.tile([P, B * L], BF16, name="kn")
    v_sb = slab.tile([P, B * 16 * 2 * 65], BF16, name="v_sb")
    V.memset(v_sb[:].rearrange("p (blk c) -> p blk c", c=65)[:, :, 64:65], 1.0)
    invk_raw = slab.tile([P, P], F32, name="invk_raw")

    qkv_loop(nc, tc, io, slab, tmp, rhsp, h1_all, mm, mm1,
             Cb, Sb, sel2, sel2T, ones128, qb_col, perm_sb, ident_sb,
             scale_sb, w_sb, qn, kn, v_sb, invk_raw)

    invk = slab.tile([P, P], F32, name="invk")
    S.activation(invk[:], invk_raw[:], AF.Sqrt)
    V.tensor_scalar_max(invk[:], invk[:], 1e-12)
    V.reciprocal_approx_fast(invk[:], invk[:])

    for name, t in (("qn", qn), ("kn", kn), ("v_sb", v_sb)):
        if name in dbg:
            dma(out=dbg[name][:], in_=t[:])

    attention(nc, tc, dbg, slab, tmp, a2a_in, mm, mm1,
              ones65, vb128, qn, kn, v_sb, invk)


def qkv_loop(nc, tc, io, slab, tmp, rhsp, h1_all, mm, mm1,
             Cb, Sb, sel2, sel2T, ones128, qb_col, perm_sb, ident_sb,
             scale_sb, w_sb, qn, kn, v_sb, invk_raw):
    V = nc.vector
    S = nc.scalar
    dma = nc.sync.dma_start
    psum_cm = tc.tile_pool(name="qkvpsum", bufs=1, space="PSUM")
    psum = psum_cm.__enter__()

    def process_q(q_ps, blk, nt):
        gsl = slice(blk * TOK + nt * 512, blk * TOK + (nt + 1) * 512)
        cpos = (blk % 2) * TOK + nt * 512
        csl = slice(cpos, cpos + 512)
        qb = tmp.tile([P, 512], BF16, name="qb")
        V.tensor_scalar_add(qb[:], q_ps[:], qb_col[:, 0:1])
        q2 = tmp.tile([P, 512], BF16, name="q2")
        V.tensor_mul(q2[:], qb[:], qb[:])
        sq_ps = psum.tile([2, 512], F32, name="sq_ps", tag="red")
        mm1(sq_ps[:], sel2[:], q2[:])
        qsd = tmp.tile([2, 512], F32, name="qsd")
        S.activation(qsd[:], sq_ps[:], AF.Sqrt)
        V.tensor_scalar_max(qsd[:], qsd[:], 1e-12)
        iq = tmp.tile([2, 512], F32, name="iq")
        V.reciprocal_approx_fast(iq[:], qsd[:])
        iq_bf = tmp.tile([2, 512], BF16, name="iq_bf")
        V.tensor_scalar_mul(iq_bf[:], iq[:], scale_sb[:, 0:1])
        swp_ps = psum.tile([P, 512], F32, name="swp_ps", tag="bcast")
        mm1(swp_ps[:], perm_sb[:], qb[:])
        t1 = tmp.tile([P, 512], BF16, name="rope_t1")
        t2 = tmp.tile([P, 512], BF16, name="rope_t2")
        V.tensor_mul(t1[:], qb[:], Cb[:, csl])
        V.tensor_mul(t2[:], swp_ps[:], Sb[:, csl])
        qr = tmp.tile([P, 512], BF16, name="qr")
        V.tensor_add(qr[:], t1[:], t2[:])
        ib_ps = psum.tile([P, 512], F32, name="ib_ps", tag="bcast")
        mm1(ib_ps[:], sel2T[:], iq_bf[:])
        V.tensor_mul(qn[:, gsl], qr[:], ib_ps[:])

    def process_k(k_ps, blk, nt):
        gsl = slice(blk * TOK + nt * 512, blk * TOK + (nt + 1) * 512)
        cpos = (blk % 2) * TOK + nt * 512
        csl = slice(cpos, cpos + 512)
        b_idx = blk // 2
        kb = tmp.tile([P, 512], BF16, name="kb")
        V.tensor_copy(kb[:], k_ps[:])
        k2 = tmp.tile([P, 512], BF16, name="k2")
        V.tensor_mul(k2[:], kb[:], kb[:])
        ks_ps = psum.tile([P, 8], F32, name="ks_ps", tag="red")
        for hh in range(2):
            for t4 in range(4):
                mm1(ks_ps[:, hh * 4 + t4:hh * 4 + t4 + 1],
                    k2[hh * 64:(hh + 1) * 64, t4 * 128:(t4 + 1) * 128],
                    ones128[hh * 64:(hh + 1) * 64, 0:1])
        kt0 = (blk % 2) * 8 + nt * 4
        base = (b_idx * 16 + kt0) * 2
        V.tensor_copy(
            invk_raw[:, base:base + 8]
            .rearrange("p (t4 h) -> p h t4", h=2),
            ks_ps[:].rearrange("p (h t4) -> p h t4", h=2))
        kswp_ps = psum.tile([P, 512], F32, name="kswp_ps", tag="bcast")
        mm1(kswp_ps[:], perm_sb[:], kb[:])
        t1 = tmp.tile([P, 512], BF16, name="rope_t1")
        t2 = tmp.tile([P, 512], BF16, name="rope_t2")
        V.tensor_mul(t1[:], kb[:], Cb[:, csl])
        V.tensor_mul(t2[:], kswp_ps[:], Sb[:, csl])
        V.tensor_add(kn[:, gsl], t1[:], t2[:])

    for blk in range(8):
        b_idx = blk // 2
        rhs = {}
        for nt in range(2):
            for ci in range(8):
                r = rhsp.tile([P, 512], BF16, name="h1r")
                dma(out=r[:], in_=h1_all[0][blk, ci * P:(ci + 1) * P,
                                            nt * 512:(nt + 1) * 512])
                rhs[(nt, ci)] = r

        # q/k accumulation; lhsT reused across nt (one LDW per 2 matmuls)
        acc = {}
        for wname in ("q", "k"):
            for nt in range(2):
                acc[(wname, nt)] = psum.tile(
                    [P, 512], F32, name=f"{wname}{nt}_ps",
                    tag=f"{wname}{nt}_ps")
        for ci in range(8):
            for w_off, wname in ((0, "q"), (128, "k")):
                wt = w_sb[:, ci * 384 + w_off:ci * 384 + w_off + 128]
                for nt in range(2):
                    mm(acc[(wname, nt)][:], wt, rhs[(nt, ci)][:],
                       start=(ci == 0), stop=(ci == 7))
        for nt in range(2):
            process_q(acc[("q", nt)], blk, nt)
            process_k(acc[("k", nt)], blk, nt)

        # v: co-major matmul then PE transpose to token-major
        for nt in range(2):
            v_ps = psum.tile([P, 512], F32, name="v_ps", tag="vtp", bufs=2)
            for ci in range(8):
                mm(v_ps[:], w_sb[:, ci * 384 + 256:ci * 384 + 384],
                   rhs[(nt, ci)][:], start=(ci == 0), stop=(ci == 7))
            vco = tmp.tile([P, 512], BF16, name="vco")
            V.tensor_copy(vco[:], v_ps[:])
            kt0 = (blk % 2) * 8 + nt * 4
            for t4 in range(4):
                tp_ps = psum.tile([P, P], BF16, name="tp_ps", tag="vtp",
                                  bufs=2)
                nc.tensor.transpose(tp_ps[:], vco[:, t4 * 128:(t4 + 1) * 128],
                                    ident_sb[:])
                kt = kt0 + t4
                vbase = (b_idx * 16 + kt) * 2 * 65
                V.tensor_copy(
                    v_sb[:, vbase:vbase + 130]
                    .rearrange("p (h c) -> p h c", h=2)[:, :, 0:64],
                    tp_ps[:].rearrange("p (h c) -> p h c", h=2))
    psum_cm.__exit__(None, None, None)


def attention(nc, tc, dbg, slab, tmp, a2a_in, mm, mm1,
              ones65, vb128, qn, kn, v_sb, invk):
    V = nc.vector
    S = nc.scalar
    dma = nc.sync.dma_start
    psum_cm = tc.tile_pool(name="atnpsum", bufs=1, space="PSUM")
    psum = psum_cm.__enter__()
    attn = slab.tile([P, B * L], BF16, name="attn")

    pending = []

    # eviction: per (b, qh, hh, j) the pv (65, 512) -> attn rows hh*64..
    def evict(b2, q2, items):
        for (hh, j, pvall) in items:
            rec = tmp.tile([65, 512], F32, name="rec")
            V.reciprocal_approx_fast(rec[:], pvall[:])
            rec_bf = tmp.tile([65, 512], BF16, name="rec_bf")
            V.tensor_copy(rec_bf[64:65, :], rec[64:65, :])
            rb_ps = psum.tile([P, 1024], F32, name="rb_ps",
                              tag="s_h0")
            mm(rb_ps[hh * 64:hh * 64 + 64, 0:512], ones65[64:65, :],
               rec_bf[64:65, :], start=True, stop=True)
            tm = tmp.tile([P, 512], BF16, name="tm")
            V.tensor_mul(tm[hh * 64:(hh + 1) * 64, :], pvall[0:64, :],
                         rb_ps[hh * 64:hh * 64 + 64, 0:512])
            col = b2 * L + q2 * 1024 + j * 512
            V.tensor_scalar_add(attn[hh * 64:(hh + 1) * 64, col:col + 512],
                                tm[hh * 64:(hh + 1) * 64, :],
                                vb128[hh * 64:(hh + 1) * 64, 0:1])

    for b_idx in range(B):
        for qh in range(2):
            pv = {}
            for hh in range(2):
                for j in range(2):
                    pv[(hh, j)] = psum.tile(
                        [65, 512], F32, name=f"pv{hh}{j}", tag=f"pv{hh}{j}")

            def drain(item):
                ktd, es = item
                for hh in range(2):
                    vb = ((b_idx * 16 + ktd) * 2 + hh) * 65
                    for j in range(2):
                        mm(pv[(hh, j)][:], v_sb[:, vb:vb + 65],
                           es[hh][:, j * 512:(j + 1) * 512],
                           start=(ktd == 0), stop=(ktd == 15))

            pend = []
            for kt in range(16):
                ksl = slice(b_idx * L + kt * 128, b_idx * L + (kt + 1) * 128)
                sh = []
                for hh in range(2):
                    s_h = psum.tile([P, 1024], F32, name=f"s_h{hh}",
                                    tag=f"s_h{hh}")
                    sh.append(s_h)
                # interleave heads so row-groups 0-63 / 64-127 overlap in PE
                for j in range(2):
                    qsl = slice(b_idx * L + qh * 1024 + j * 512,
                                b_idx * L + qh * 1024 + (j + 1) * 512)
                    for hh in range(2):
                        hs = slice(hh * 64, (hh + 1) * 64)
                        mm1(sh[hh][:, j * 512:(j + 1) * 512],
                            kn[hs, ksl], qn[hs, qsl])
                es = []
                for hh in range(2):
                    e_bf = tmp.tile([P, 1024], BF16, name="e_bf", bufs=6)
                    ikcol = (b_idx * 16 + kt) * 2 + hh
                    S.activation(e_bf[:], sh[hh][:], AF.Exp,
                                 scale=invk[:, ikcol:ikcol + 1])
                    es.append(e_bf)
                pend.append((kt, es))
                if len(pend) > 2:
                    drain(pend.pop(0))
                if kt in (6, 8, 10, 12) and pending:
                    b2, q2, items = pending[0]
                    evict(b2, q2, [items.pop(0)])
                    if not items:
                        pending.pop(0)
            for item in pend:
                drain(item)

            items = []
            for hh in range(2):
                for j in range(2):
                    pvall = tmp.tile([65, 512], F32, name="pvall", bufs=8)
                    V.tensor_copy(pvall[:], pv[(hh, j)][:])
                    items.append((hh, j, pvall))
            pending.append((b_idx, qh, items))
    while pending:
        b2, q2, items = pending.pop(0)
        evict(b2, q2, items)

    psum_cm.__exit__(None, None, None)
    if "attn" in dbg:
        dma(out=dbg["attn"][:], in_=attn[:])
    dma(out=a2a_in[:].rearrange("blk p t -> p blk t"),
        in_=attn[:].rearrange("p (blk t) -> p blk t", blk=8))


# ---------------------------------------------------------------------------
# host-side input preparation
# ---------------------------------------------------------------------------

_PERM = np.concatenate([np.arange(0, HD, 2), np.arange(1, HD, 2)])  # re|im


def _perm_matrix():
    """(128,128) with entry (swap(m), m) = 1; swap exchanges the re (0:32)
    and im (32:64) halves of each 64-row head slice."""
    pm = np.zeros((P, P), np.float32)
    for m in range(P):
        base = (m // 64) * 64
        r = m - base
        sw = base + (r + 32) % 64
        pm[sw, m] = 1.0
    return pm


def prep_in_maps(inputs):
    import ml_dtypes
    bf = lambda a: np.ascontiguousarray(a).astype(ml_dtypes.bfloat16)
    f32 = lambda a: np.ascontiguousarray(np.asarray(a, dtype=np.float32))

    x = np.asarray(inputs["x"], np.float32)
    cond = np.asarray(inputs["cond_BD"], np.float32)
    W_qkv = np.asarray(inputs["W_qkv"], np.float32)
    q_bias = np.asarray(inputs["q_bias"], np.float32)
    v_bias = np.asarray(inputs["v_bias"], np.float32)
    sml = np.asarray(inputs["scale_mul_log"], np.float32).reshape(H)
    W_proj = np.asarray(inputs["W_proj"], np.float32)
    b_proj = np.asarray(inputs["b_proj"], np.float32)
    W_fc1 = np.asarray(inputs["W_fc1"], np.float32)
    b_fc1 = np.asarray(inputs["b_fc1"], np.float32)
    W_fc2 = np.asarray(inputs["W_fc2"], np.float32)
    b_fc2 = np.asarray(inputs["b_fc2"], np.float32)
    W_ada = np.asarray(inputs["W_ada"], np.float32)
    b_ada = np.asarray(inputs["b_ada"], np.float32)
    fc = np.asarray(inputs["freqs_cos"], np.float32)
    fs = np.asarray(inputs["freqs_sin"], np.float32)

    cosT = f32(fc.T)   # (32, L)
    sinT = f32(fs.T)
    w_projT = bf(W_proj.T)
    f8 = lambda a: np.ascontiguousarray(a).astype(ml_dtypes.float8_e4m3)
    w_fc1T = f8(W_fc1.T * 64.0)
    w_fc2T = f8(W_fc2.T * 64.0)
    b_proj_c = f32(b_proj.reshape(8, P).T)
    b_fc1_c = f32(b_fc1.reshape(32, P).T)
    b_fc2_c = f32(b_fc2.reshape(8, P).T * 64.0)
    w_adaT = W_ada.T  # (1024, 6144)

    in_maps = []
    for m in range(N_CORES):
        b_own, pm = m // 2, m % 2
        h0, h1 = 2 * m, 2 * m + 1
        cols = []
        for h in (h0, h1):
            cols.append(W_qkv[h * HD + _PERM, :])          # q rows, permuted
        for h in (h0, h1):
            cols.append(W_qkv[C + h * HD + _PERM, :])      # k rows, permuted
        for h in (h0, h1):
            cols.append(W_qkv[2 * C + h * HD:2 * C + (h + 1) * HD, :])  # v
        w_qkv_m = bf(np.concatenate(cols, axis=0).T)       # (1024, 384)
        qkv_b_m = np.concatenate([
            q_bias[h0 * HD + _PERM], q_bias[h1 * HD + _PERM],
            np.zeros(P, np.float32),
            v_bias[h0 * HD:(h0 + 1) * HD], v_bias[h1 * HD:(h1 + 1) * HD],
        ]).reshape(384, 1)

        vb2 = np.stack([v_bias[h0 * HD:(h0 + 1) * HD],
                        v_bias[h1 * HD:(h1 + 1) * HD]], axis=1)
        vb128 = np.concatenate([v_bias[h0 * HD:(h0 + 1) * HD],
                                v_bias[h1 * HD:(h1 + 1) * HD]]).reshape(P, 1)
        xm = x[b_own, pm * TOK:(pm + 1) * TOK, :]          # (1024, 1024)
        identm = np.eye(P, dtype=np.float32)
        s2t = np.zeros((2, P), np.float32)
        s2t[0, 0:64] = 1.0
        s2t[1, 64:128] = 1.0
        in_maps.append({
            "perm_m": bf(_perm_matrix()),
            "sel2t": bf(s2t),
            "ident": bf(identm),
            "vb2": f32(vb2),
            "vb128": f32(vb128),
            "xT": f32(xm.T),
            "condT": f32(cond[b_own].reshape(C, 1)),
            "w_ada": bf(w_adaT[:, pm * 3 * C:(pm + 1) * 3 * C]),
            "b_ada_r": f32(b_ada[pm * 3 * C:(pm + 1) * 3 * C].reshape(1, -1)),
            "w_qkv": w_qkv_m,
            "qkv_b": f32(qkv_b_m),
            "scale_log": f32(sml[[h0, h1]].reshape(2, 1)),
            "cosT": cosT, "sinT": sinT,
            "w_proj": w_projT, "b_proj_c": b_proj_c,
            "w_fc1": w_fc1T, "b_fc1_c": b_fc1_c,
            "w_fc2": w_fc2T, "b_fc2_c": b_fc2_c,
        })
    return in_maps


_NC_CACHE = {}


def _get_nc(debug_outputs=()):
    key = tuple(sorted(debug_outputs))
    if key not in _NC_CACHE:
        _NC_CACHE[key] = build_nc(debug_outputs)
    return _NC_CACHE[key]


def run(inputs, debug_outputs=(), trace=False):
    nc = _get_nc(debug_outputs)
    in_maps = prep_in_maps(inputs)
    res = run_bass_kernel_spmd(nc, in_maps, core_ids=list(range(N_CORES)),
                               trace=trace)
    out = np.empty((B, L, C), np.float32)
    for m in range(N_CORES):
        b_own, pm = m // 2, m % 2
        out[b_own, pm * TOK:(pm + 1) * TOK, :] = res.results[m]["outT"].T
    return out, res


def kernel(**inputs):
    out, _ = run(inputs)
    return out


# revision 35
# speedup vs baseline: 1.2688x; 1.1229x over previous
"""Trainium2 Bass kernel for nn_AdaLNSelfAttn_RoPE (B=4, L=2048, C=1024, H=16).

Sharding across 8 NeuronCores (one chip):
  - Tokens flattened (B*L = 8192) and sharded 1024/core; each core owns half of
    one batch.  LayerNorms, AdaLN modulation, proj, and the FFN run on the
    token shard (sequence parallel).
  - QKV + attention are head-sharded: core m owns heads {2m, 2m+1}.  The
    modulated hidden h1 is AllGather'ed (bf16) so every core computes QKV for
    its two heads over all tokens.  Attention outputs return to token shards
    via AllToAll.
  - AdaLN (silu+linear) is column-sharded across pairs of cores (both cores of
    a pair own the same batch) with a 2-wide AllGather.
  - Activations are channel-major (C on partitions) so matmuls chain without
    transposes; per-token statistics (LN mean/rstd, softmax rowsums, q-norms)
    are reduced across partitions with ones-matmuls and broadcast back with
    outer-product matmuls.
  - Matmuls in bf16 (fp32 PSUM accumulation); statistics in fp32.
  - Softmax skips max-subtraction (cosine attention bounds scores to [-4,4]).
    The softmax denominator rides as a 65th output column of the PV matmul;
    the k-side 1/||k|| rides as the per-partition scale of the exp()
    activation.
  - RoPE pairs are pre-permuted on the host (re parts in rows 0-31 of each
    head slice, im parts in rows 32-63) so rotation is four 32-row block ops.
"""

import numpy as np

import concourse.bass as bass
import concourse.bacc as bacc
import concourse.mybir as mybir
import concourse.tile as tile
from concourse.bass_utils import run_bass_kernel_spmd

F32 = mybir.dt.float32
BF16 = mybir.dt.bfloat16
FP8 = mybir.dt.float8e4
W8SCALE = 64.0
AF = mybir.ActivationFunctionType
ALU = mybir.AluOpType

B, L, C, H = 4, 2048, 1024, 16
HD = C // H          # 64
N_CORES = 8
TOK = (B * L) // N_CORES   # 1024 tokens per core
MAX_SCALE = float(np.log(100.0))
LN_EPS = 1e-6
P = 128


def build_nc(debug_outputs=()):
    nc = bacc.Bacc("TRN2", target_bir_lowering=False, debug=False,
                   num_devices=N_CORES)

    dt = nc.dram_tensor
    io = {}
    io["xT"] = dt("xT", [C, TOK], F32, kind="ExternalInput")
    io["condT"] = dt("condT", [C, 1], F32, kind="ExternalInput")
    io["w_ada"] = dt("w_ada", [C, 3 * C], BF16, kind="ExternalInput")
    io["b_ada_r"] = dt("b_ada_r", [1, 3 * C], F32, kind="ExternalInput")
    io["w_qkv"] = dt("w_qkv", [C, 384], BF16, kind="ExternalInput")
    io["qkv_b"] = dt("qkv_b", [384, 1], F32, kind="ExternalInput")
    io["perm_m"] = dt("perm_m", [P, P], BF16, kind="ExternalInput")
    io["sel2t"] = dt("sel2t", [2, P], BF16, kind="ExternalInput")
    io["ident"] = dt("ident", [P, P], BF16, kind="ExternalInput")
    io["vb2"] = dt("vb2", [64, 2], F32, kind="ExternalInput")
    io["vb128"] = dt("vb128", [P, 1], F32, kind="ExternalInput")
    io["scale_log"] = dt("scale_log", [2, 1], F32, kind="ExternalInput")
    io["cosT"] = dt("cosT", [32, L], F32, kind="ExternalInput")
    io["sinT"] = dt("sinT", [32, L], F32, kind="ExternalInput")
    io["w_proj"] = dt("w_proj", [C, C], BF16, kind="ExternalInput")
    io["b_proj_c"] = dt("b_proj_c", [P, 8], F32, kind="ExternalInput")
    io["w_fc1"] = dt("w_fc1", [C, 4 * C], FP8, kind="ExternalInput")
    io["b_fc1_c"] = dt("b_fc1_c", [P, 32], F32, kind="ExternalInput")
    io["w_fc2"] = dt("w_fc2", [4 * C, C], FP8, kind="ExternalInput")
    io["b_fc2_c"] = dt("b_fc2_c", [P, 8], F32, kind="ExternalInput")
    io["outT"] = dt("outT", [C, TOK], F32, kind="ExternalOutput")

    dbg = {}
    for name, shape, dtp in [
        ("h1_all", [8, C, TOK], BF16),
        ("qn", [P, B * L], BF16),
        ("kn", [P, B * L], BF16),
        ("v_sb", [P, B * 16 * 2 * 65], BF16),
        ("attn", [P, B * L], BF16),
        ("a2a_out", [8, P, TOK], BF16),
        ("x2", [P, 8 * TOK], F32),
        ("ada_all", [2, 3 * C], F32),
    ]:
        if name in debug_outputs:
            dbg[name] = dt("dbg_" + name, shape, dtp, kind="ExternalOutput")

    with tile.TileContext(nc) as tc:
        _body(nc, tc, io, dbg)
    nc.compile()
    return nc


def _body(nc, tc, io, dbg):
    mm = nc.tensor.matmul
    V = nc.vector
    S = nc.scalar
    dma = nc.sync.dma_start

    def mm1(out, lhsT, rhs):
        mm(out, lhsT, rhs, start=True, stop=True)

    with tc.tile_pool(name="dram", bufs=1, space="DRAM") as dram, \
         tc.tile_pool(name="const", bufs=1) as const, \
         tc.tile_pool(name="outer", bufs=1) as outer:

        # DRAM bounce buffers for collectives
        ada_in = dram.tile([1, 3 * C], F32, name="ada_in")
        ada_all = dram.tile([2, 3 * C], F32, name="ada_all")
        h1_in = dram.tile([TOK, C], BF16, name="h1_in")
        h1_all = [dram.tile([8, C, TOK], BF16, addr_space="Shared",
                            name="h1_all0")]
        a2a_in = dram.tile([8, P, TOK], BF16, name="a2a_in")
        a2a_out = dram.tile([8, P, TOK], BF16, name="a2a_out")

        # ------------------------------------------------------- constants
        ones128 = const.tile([P, 1], BF16, name="ones128")
        V.memset(ones128[:], 1.0)
        ones64c = const.tile([64, 1], BF16, name="ones64c")
        V.memset(ones64c[:], 1.0)
        ones1x64 = const.tile([1, 64], BF16, name="ones1x64")
        V.memset(ones1x64[:], 1.0)
        sel2 = const.tile([P, 2], BF16, name="sel2")     # head indicator cols
        V.memset(sel2[:], 0.0)
        V.memset(sel2[0:64, 0:1], 1.0)
        V.memset(sel2[64:128, 1:2], 1.0)
        sel2T = const.tile([2, P], BF16, name="sel2T")   # head indicator rows
        dma(out=sel2T[:], in_=io["sel2t"][:, :])

        # rope cos (128, 2048) bf16: 4 vertical copies of (32, 2048); and
        # sign-baked sin: rows [0:32]=-sin [32:64]=+sin [64:96]=-sin [96:]=+sin
        Cb = const.tile([P, L], BF16, name="Cb")
        Sb = const.tile([P, L], BF16, name="Sb")
        with tc.tile_pool(name="cs_pool", bufs=1) as csp:
            cs_f = csp.tile([P, L], F32, name="cs_f")
            for j in range(4):
                dma(out=cs_f[32 * j:32 * (j + 1), :], in_=io["cosT"][:, :])
            V.tensor_copy(Cb[:], cs_f[:])
            sn_f = csp.tile([P, L], F32, name="sn_f")
            for j in range(4):
                dma(out=sn_f[32 * j:32 * (j + 1), :], in_=io["sinT"][:, :])
            for j in range(4):
                sgn = -1.0 if j % 2 == 0 else 1.0
                S.activation(Sb[32 * j:32 * (j + 1), :],
                             sn_f[32 * j:32 * (j + 1), :], AF.Copy, scale=sgn)
        perm_sb = const.tile([P, P], BF16, name="perm_sb")
        dma(out=perm_sb[:], in_=io["perm_m"][:, :])
        ident_sb = const.tile([P, P], BF16, name="ident_sb")
        dma(out=ident_sb[:], in_=io["ident"][:, :])
        vb2 = const.tile([64, 2], F32, name="vb2")
        dma(out=vb2[:], in_=io["vb2"][:, :])
        vb128 = const.tile([P, 1], F32, name="vb128")
        dma(out=vb128[:], in_=io["vb128"][:, :])
        ones65 = const.tile([65, 64], BF16, name="ones65")
        V.memset(ones65[:], 1.0)

        # bias columns
        qb_col = const.tile([P, 3], F32, name="qb_col")
        dma(out=qb_col[:], in_=io["qkv_b"].ap().rearrange("(m p) 1 -> p m", p=P))
        bproj = const.tile([P, 8], F32, name="bproj")
        dma(out=bproj[:], in_=io["b_proj_c"][:, :])
        bfc1 = const.tile([P, 32], F32, name="bfc1")
        dma(out=bfc1[:], in_=io["b_fc1_c"][:, :])
        bfc2 = const.tile([P, 8], F32, name="bfc2")
        dma(out=bfc2[:], in_=io["b_fc2_c"][:, :])
        bada = const.tile([1, 3 * C], F32, name="bada")
        dma(out=bada[:], in_=io["b_ada_r"][:, :])
        epsc = const.tile([1, 1], F32, name="epsc")
        V.memset(epsc[:], LN_EPS)

        # scale_mul = exp(min(scale_log, MAX_SCALE))
        scale_sb = const.tile([2, 1], F32, name="scale_sb")
        with tc.tile_pool(name="scp", bufs=1) as scp:
            sc_raw = scp.tile([2, 1], F32, name="sc_raw")
            dma(out=sc_raw[:], in_=io["scale_log"][:, :])
            sc_min = scp.tile([2, 1], F32, name="sc_min")
            V.tensor_scalar_min(sc_min[:], sc_raw[:], MAX_SCALE)
            S.activation(scale_sb[:], sc_min[:], AF.Exp)

        # --------------------------------------------------- phase 0: adaLN
        ada_phase(nc, tc, io, dram, const, ada_in, ada_all)

        g1c = ada_col(nc, const, ada_all, 0, "g1c")
        g2c = ada_col(nc, const, ada_all, 1, "g2c")
        sh1c = ada_col(nc, const, ada_all, 4, "sh1c")
        sh2c = ada_col(nc, const, ada_all, 5, "sh2c")
        s1p = ada_srow(nc, tc, const, ada_all, 2, "s1p")
        s2p = ada_srow(nc, tc, const, ada_all, 3, "s2p")

        if "ada_all" in dbg:
            dma(out=dbg["ada_all"][:], in_=ada_all[:])

        # ------------------------------------------------------ LN1 -> AG h1
        with tc.tile_pool(name="ln1pool", bufs=1) as lp, \
             tc.tile_pool(name="ln1psum", bufs=2, space="PSUM") as lps, \
             tc.tile_pool(name="ln1tmp", bufs=3) as ltmp:
            xT_sb = lp.tile([P, 8 * TOK], F32, name="xT_sb")
            dma(out=xT_sb[:].rearrange("p (blk t) -> p blk t", blk=8),
                in_=io["xT"].ap().rearrange("(blk p) t -> p blk t", p=P))
            h1_bf = lp.tile([P, 8 * TOK], BF16, name="h1_bf")

            def h1_chunk_out(ci2):
                # stage channel blocks as they complete; one AG at the end
                csl = slice(2 * ci2 * P, 2 * (ci2 + 1) * P)
                dma(out=h1_in[csl, :].rearrange("(blk p) t -> p blk t", p=P),
                    in_=h1_bf[:, 2 * ci2 * TOK:2 * (ci2 + 1) * TOK]
                    .rearrange("p (blk t) -> p blk t", blk=2))

            layer_norm(nc, lp, lps, ltmp, xT_sb, s1p, sh1c, h1_bf,
                       ones128, epsc, mm, block_done=h1_chunk_out)
        nc.gpsimd.collective_compute(
            "AllGather", ALU.bypass,
            replica_groups=[list(range(N_CORES))],
            ins=[h1_in[:].opt()], outs=[h1_all[0][:].opt()])
        if "h1_all" in dbg:
            dma(out=dbg["h1_all"][:], in_=h1_all[0][:])

        # ------------------------------------- phase 2+3: qkv, rope, attention
        with tc.tile_pool(name="atslab", bufs=1) as ats, \
             tc.tile_pool(name="attmp", bufs=2) as atmp, \
             tc.tile_pool(name="atrhs", bufs=18) as arhs:
            qkv_phase(nc, tc, io, dbg, ats, atmp, arhs,
                      h1_all, a2a_in, mm, mm1,
                      Cb, Sb, sel2, sel2T, ones128, ones65,
                      qb_col, vb128, perm_sb, ident_sb, scale_sb)
        nc.gpsimd.collective_compute(
            "AllToAll", ALU.bypass,
            replica_groups=[list(range(N_CORES))],
            ins=[a2a_in[:].opt()], outs=[a2a_out[:].opt()])
        if "a2a_out" in dbg:
            dma(out=dbg["a2a_out"][:], in_=a2a_out[:])

        # ----------------------------------------- phase 4: proj + residual
        x2p_cm = tc.tile_pool(name="x2pool", bufs=1)
        x2p = x2p_cm.__enter__()
        x2_sb = x2p.tile([P, 8 * TOK], F32, name="x2_sb")
        with tc.tile_pool(name="pjpool", bufs=1) as pjp, \
             tc.tile_pool(name="pjpsum", bufs=2, space="PSUM") as pjps, \
             tc.tile_pool(name="pjtmp", bufs=3) as ptmp:
            wp_sb = pjp.tile([P, 8 * C], BF16, name="wp_sb")
            dma(out=wp_sb[:].rearrange("p (blk c) -> p blk c", blk=8),
                in_=io["w_proj"].ap().rearrange("(blk p) c -> p blk c", p=P))
            prhs = []
            for r in range(8):
                for nt in range(2):
                    t = pjp.tile([P, 512], BF16, name=f"prhs_{r}_{nt}",
                                 tag="prhs", bufs=16)
                    dma(out=t[:], in_=a2a_out[r, :, nt * 512:(nt + 1) * 512])
                    prhs.append(t)
            for cot in range(8):
                xres = ptmp.tile([P, 2 * 512], F32, name="xres")
                dma(out=xres[:],
                    in_=io["xT"][cot * P:(cot + 1) * P, :])
                pp = [pjps.tile([P, 512], F32, name=f"p_ps{nt}",
                                tag=f"p_ps{nt}") for nt in range(2)]
                for r in range(8):
                    wt = wp_sb[:, r * C + cot * P:r * C + (cot + 1) * P]
                    for nt in range(2):
                        mm(pp[nt][:], wt, prhs[r * 2 + nt][:],
                           start=(r == 0), stop=(r == 7))
                for nt in range(2):
                    t1 = ptmp.tile([P, 512], F32, name="pj_t1")
                    V.tensor_scalar(t1[:], pp[nt][:], bproj[:, cot:cot + 1],
                                    g1c[:, cot:cot + 1], ALU.add, ALU.mult)
                    sl = slice(cot * TOK + nt * 512, cot * TOK + (nt + 1) * 512)
                    V.tensor_add(x2_sb[:, sl], t1[:],
                                 xres[:, nt * 512:(nt + 1) * 512])
        if "x2" in dbg:
            dma(out=dbg["x2"][:], in_=x2_sb[:])

        # ------------------------------------------------- phase 5-7: LN2+FFN
        with tc.tile_pool(name="ffnpool", bufs=1) as fp, \
             tc.tile_pool(name="ffntmp", bufs=3) as ftmp:
            h2_bf = fp.tile([P, 8 * TOK], FP8, name="h2_bf")
            with tc.tile_pool(name="ln2pool", bufs=1) as lp2, \
                 tc.tile_pool(name="ln2psum", bufs=2, space="PSUM") as lps2:
                layer_norm(nc, lp2, lps2, ftmp, x2_sb, s2p, sh2c, h2_bf,
                           ones128, epsc, mm)

            g2s = ftmp.tile([P, 8], F32, name="g2s", bufs=1)
            V.tensor_scalar_mul(g2s[:], g2c[:], 1.0 / W8SCALE)
            fw_cm = tc.tile_pool(name="ffnw", bufs=2)
            fw = fw_cm.__enter__()
            fps_cm = tc.tile_pool(name="ffnpsum", bufs=4, space="PSUM")
            fps = fps_cm.__enter__()
            hact = fp.tile([P, 32 * TOK], FP8, name="hact")
            h2v = h2_bf[:].rearrange("p (blk t) -> p blk t", blk=8)
            for cot in range(32):
                w1 = fw.tile([P, 8 * P], FP8, name="w1")
                dma(out=w1[:].rearrange("p (blk c) -> p blk c", blk=8),
                    in_=io["w_fc1"].ap()[:, cot * P:(cot + 1) * P]
                    .rearrange("(blk p) c -> p blk c", p=P))
                w1v = w1[:].rearrange("p (ki j c) -> p ki j c", ki=4, j=2)
                fpp = [fps.tile([P, 512], F32, name=f"f_ps{nt}",
                                tag=f"f_ps{nt}", bufs=2) for nt in range(2)]
                for ki in range(4):
                    wt = w1v[:, ki, :, :]
                    for nt in range(2):
                        rhs3 = h2v[:, 2 * ki:2 * ki + 2,
                                   nt * 512:(nt + 1) * 512]
                        mm(fpp[nt][:], wt, rhs3,
                           perf_mode=mybir.MatmulPerfMode.DoubleRow,
                           start=(ki == 0), stop=(ki == 3))
                for nt in range(2):
                    S.activation(
                        hact[:, cot * TOK + nt * 512:cot * TOK + (nt + 1) * 512],
                        fpp[nt][:], AF.Gelu_apprx_tanh,
                        bias=bfc1[:, cot:cot + 1], scale=1.0 / W8SCALE)

            hactv = hact[:].rearrange("p (blk t) -> p blk t", blk=32)
            for cot in range(8):
                w2 = fw.tile([P, 32 * P], FP8, name="w2")
                dma(out=w2[:].rearrange("p (blk c) -> p blk c", blk=32),
                    in_=io["w_fc2"].ap()[:, cot * P:(cot + 1) * P]
                    .rearrange("(blk p) c -> p blk c", p=P))
                w2v = w2[:].rearrange("p (ki j c) -> p ki j c", ki=16, j=2)
                opp = [fps.tile([P, 512], F32, name=f"o_ps{nt}",
                                tag=f"o_ps{nt}", bufs=2) for nt in range(2)]
                for ki in range(16):
                    wt = w2v[:, ki, :, :]
                    for nt in range(2):
                        rhs3 = hactv[:, 2 * ki:2 * ki + 2,
                                     nt * 512:(nt + 1) * 512]
                        mm(opp[nt][:], wt, rhs3,
                           perf_mode=mybir.MatmulPerfMode.DoubleRow,
                           start=(ki == 0), stop=(ki == 15))
                for nt in range(2):
                    t1 = ftmp.tile([P, 512], F32, name="o_t1")
                    V.tensor_scalar(t1[:], opp[nt][:], bfc2[:, cot:cot + 1],
                                    g2s[:, cot:cot + 1], ALU.add, ALU.mult)
                    ot = ftmp.tile([P, 512], F32, name="ot")
                    sl = slice(cot * TOK + nt * 512, cot * TOK + (nt + 1) * 512)
                    V.tensor_add(ot[:], t1[:], x2_sb[:, sl])
                    dma(out=io["outT"][cot * P:(cot + 1) * P,
                                       nt * 512:(nt + 1) * 512],
                        in_=ot[:])
            fps_cm.__exit__(None, None, None)
            fw_cm.__exit__(None, None, None)
        x2p_cm.__exit__(None, None, None)


def ada_phase(nc, tc, io, dram, const, ada_in, ada_all):
    """silu(cond) @ W_ada_slice.T + b_ada, pair-wise AllGather."""
    mm = nc.tensor.matmul
    V = nc.vector
    S = nc.scalar
    dma = nc.sync.dma_start
    cond_sb = const.tile([P, 8], F32, name="cond_sb")
    dma(out=cond_sb[:],
        in_=io["condT"].ap().rearrange("(blk p) 1 -> p blk", p=P))
    scond = const.tile([P, 8], BF16, name="scond")
    S.activation(scond[:], cond_sb[:], AF.Silu)
    bada = const.tile([1, 3 * C], F32, name="bada2")
    dma(out=bada[:], in_=io["b_ada_r"][:, :])

    with tc.tile_pool(name="adapool", bufs=1) as ap_, \
         tc.tile_pool(name="adapsum", bufs=2, space="PSUM") as aps, \
         tc.tile_pool(name="adarhs", bufs=4) as arp:
        ada_sb = ap_.tile([1, 3 * C], F32, name="ada_sb")
        for nt6 in range(6):
            a_ps = aps.tile([1, 512], F32, name="a_ps")
            for ci in range(8):
                wt = arp.tile([P, 512], BF16, name="wt_ada")
                dma(out=wt[:], in_=io["w_ada"][ci * P:(ci + 1) * P,
                                               nt6 * 512:(nt6 + 1) * 512])
                mm(a_ps[:], scond[:, ci:ci + 1], wt[:],
                   start=(ci == 0), stop=(ci == 7))
            V.tensor_add(ada_sb[0:1, nt6 * 512:(nt6 + 1) * 512], a_ps[:],
                         bada[0:1, nt6 * 512:(nt6 + 1) * 512])
        dma(out=ada_in[:], in_=ada_sb[:])
    nc.gpsimd.collective_compute(
        "AllGather", ALU.bypass,
        replica_groups=[[0, 1], [2, 3], [4, 5], [6, 7]],
        ins=[ada_in[:].opt()], outs=[ada_all[:].opt()])


def ada_col(nc, const, ada_all, vec, name):
    r, off = (vec * C) // (3 * C), (vec * C) % (3 * C)
    t = const.tile([P, 8], F32, name=name)
    nc.sync.dma_start(out=t[:], in_=ada_all[r:r + 1, off:off + C]
                      .rearrange("1 (blk p) -> p blk", p=P))
    return t


def ada_srow(nc, tc, const, ada_all, vec, name):
    """(1, 1024) bf16 row of (ada_vec + 1)."""
    r, off = (vec * C) // (3 * C), (vec * C) % (3 * C)
    t = const.tile([1, C], BF16, name=name)
    with tc.tile_pool(name=name + "_f", bufs=1) as p:
        raw = p.tile([1, C], F32, name=name + "_raw")
        nc.sync.dma_start(out=raw[:], in_=ada_all[r:r + 1, off:off + C])
        nc.vector.tensor_scalar_add(t[:], raw[:], 1.0)
    return t


def layer_norm(nc, pool, psum, tmp, x_sb, sp_row, sh_col, out_bf, ones128, epsc, mm,
               block_done=None):
    """x_sb (128, 8192) f32, channel-major blocks; out_bf same layout bf16:
    LN(x) * (s+1) + sh, statistics over the channel (partition x block) dim."""
    V = nc.vector
    S = nc.scalar
    xc = pool.tile([P, 8 * TOK], BF16, name="ln_xc")
    for ci in range(8):
        sl = slice(ci * TOK, (ci + 1) * TOK)
        V.tensor_copy(xc[:, sl], x_sb[:, sl])
    mu_ps = [psum.tile([1, 512], F32, name=f"mu_ps{nt}", tag=f"mu_ps{nt}",
                       bufs=1) for nt in range(2)]
    s2_ps = [psum.tile([1, 512], F32, name=f"s2_ps{nt}", tag=f"s2_ps{nt}",
                       bufs=1) for nt in range(2)]
    for ci in range(8):
        sl = slice(ci * TOK, (ci + 1) * TOK)
        xsq = tmp.tile([P, TOK], BF16, name="ln_xsq")
        S.activation(xsq[:], xc[:, sl], AF.Square)
        for nt in range(2):
            tsl = slice(ci * TOK + nt * 512, ci * TOK + (nt + 1) * 512)
            mm(mu_ps[nt][:], ones128[:], xc[:, tsl],
               start=(ci == 0), stop=(ci == 7))
            mm(s2_ps[nt][:], ones128[:], xsq[:, nt * 512:(nt + 1) * 512],
               start=(ci == 0), stop=(ci == 7))
    mu = pool.tile([1, TOK], F32, name="ln_mu")
    va = pool.tile([1, TOK], F32, name="ln_va")
    for nt in range(2):
        tsl = slice(nt * 512, (nt + 1) * 512)
        S.activation(mu[0:1, tsl], mu_ps[nt][:], AF.Copy, scale=1.0 / C)
        S.activation(va[0:1, tsl], s2_ps[nt][:], AF.Copy, scale=1.0 / C)
    # va := rstd = 1/sqrt(va - mu^2 + eps), in place
    mu2 = pool.tile([1, TOK], F32, name="ln_mu2")
    V.tensor_mul(mu2[:], mu[:], mu[:])
    V.tensor_sub(va[:], va[:], mu2[:])
    S.activation(va[:], va[:], AF.Sqrt, bias=epsc[0:1, 0:1])
    V.reciprocal_approx_fast(va[:], va[:])
    rm = pool.tile([1, TOK], F32, name="ln_rm")
    V.tensor_mul(rm[:], va[:], mu[:])
    rstd_bf = pool.tile([1, TOK], BF16, name="ln_rstd_bf")
    V.tensor_copy(rstd_bf[:], va[:])
    rm_bf = pool.tile([1, TOK], BF16, name="ln_rm_bf")
    V.tensor_copy(rm_bf[:], rm[:])

    for ci in range(8):
        for nt in range(2):
            a_ps = psum.tile([P, 512], F32, name="lnA_ps")
            b_ps = psum.tile([P, 512], F32, name="lnB_ps")
            tsl = slice(nt * 512, (nt + 1) * 512)
            mm(a_ps[:], sp_row[0:1, ci * P:(ci + 1) * P], rstd_bf[0:1, tsl],
               start=True, stop=True)
            mm(b_ps[:], sp_row[0:1, ci * P:(ci + 1) * P], rm_bf[0:1, tsl],
               start=True, stop=True)
            sl = slice(ci * TOK + nt * 512, ci * TOK + (nt + 1) * 512)
            t1 = tmp.tile([P, 512], BF16, name="ln_t1")
            V.tensor_mul(t1[:], xc[:, sl], a_ps[:])
            V.scalar_tensor_tensor(out_bf[:, sl], t1[:],
                                   sh_col[:, ci:ci + 1], b_ps[:],
                                   ALU.add, ALU.subtract)
        if block_done is not None and ci % 2 == 1:
            block_done(ci // 2)


def qkv_phase(nc, tc, io, dbg, slab, tmp, rhsp,
              h1_all, a2a_in, mm, mm1,
              Cb, Sb, sel2, sel2T, ones128, ones65, qb_col, vb128, perm_sb,
              ident_sb, scale_sb):
    V = nc.vector
    S = nc.scalar
    dma = nc.sync.dma_start

    w_sb = slab.tile([P, 8 * 384], BF16, name="w_sb")
    dma(out=w_sb[:].rearrange("p (blk c) -> p blk c", blk=8),
        in_=io["w_qkv"].ap().rearrange("(blk p) c -> p blk c", p=P))

    qn = slab.tile([P, B * L], BF16, name="qn")      # (128, 8192)
    kn = slab.tile([P, B * L], BF16, name="kn")
    v_sb = slab.tile([P, B * 16 * 2 * 65], BF16, name="v_sb")
    V.memset(v_sb[:].rearrange("p (blk c) -> p blk c", c=65)[:, :, 64:65], 1.0)
    invk_raw = slab.tile([P, P], F32, name="invk_raw")

    qkv_loop(nc, tc, io, slab, tmp, rhsp, h1_all, mm, mm1,
             Cb, Sb, sel2, sel2T, ones128, qb_col, perm_sb, ident_sb,
             scale_sb, w_sb, qn, kn, v_sb, invk_raw)

    invk = slab.tile([P, P], F32, name="invk")
    S.activation(invk[:], invk_raw[:], AF.Sqrt)
    V.tensor_scalar_max(invk[:], invk[:], 1e-12)
    V.reciprocal_approx_fast(invk[:], invk[:])

    for name, t in (("qn", qn), ("kn", kn), ("v_sb", v_sb)):
        if name in dbg:
            dma(out=dbg[name][:], in_=t[:])

    attention(nc, tc, dbg, slab, tmp, a2a_in, mm, mm1,
              ones65, vb128, qn, kn, v_sb, invk)


def qkv_loop(nc, tc, io, slab, tmp, rhsp, h1_all, mm, mm1,
             Cb, Sb, sel2, sel2T, ones128, qb_col, perm_sb, ident_sb,
             scale_sb, w_sb, qn, kn, v_sb, invk_raw):
    V = nc.vector
    S = nc.scalar
    dma = nc.sync.dma_start
    psum_cm = tc.tile_pool(name="qkvpsum", bufs=1, space="PSUM")
    psum = psum_cm.__enter__()

    def process_q(q_ps, blk, nt):
        gsl = slice(blk * TOK + nt * 512, blk * TOK + (nt + 1) * 512)
        cpos = (blk % 2) * TOK + nt * 512
        csl = slice(cpos, cpos + 512)
        qb = tmp.tile([P, 512], BF16, name="qb")
        V.tensor_scalar_add(qb[:], q_ps[:], qb_col[:, 0:1])
        q2 = tmp.tile([P, 512], BF16, name="q2")
        S.activation(q2[:], qb[:], AF.Square)
        sq_ps = psum.tile([2, 512], F32, name="sq_ps", tag="red")
        mm1(sq_ps[:], sel2[:], q2[:])
        qsd = tmp.tile([2, 512], F32, name="qsd")
        S.activation(qsd[:], sq_ps[:], AF.Sqrt)
        V.tensor_scalar_max(qsd[:], qsd[:], 1e-12)
        iq = tmp.tile([2, 512], F32, name="iq")
        V.reciprocal_approx_fast(iq[:], qsd[:])
        iq_bf = tmp.tile([2, 512], BF16, name="iq_bf")
        V.tensor_scalar_mul(iq_bf[:], iq[:], scale_sb[:, 0:1])
        swp_ps = psum.tile([P, 512], F32, name="swp_ps", tag="bcast")
        mm1(swp_ps[:], perm_sb[:], qb[:])
        t1 = tmp.tile([P, 512], BF16, name="rope_t1")
        t2 = tmp.tile([P, 512], BF16, name="rope_t2")
        V.tensor_mul(t1[:], qb[:], Cb[:, csl])
        V.tensor_mul(t2[:], swp_ps[:], Sb[:, csl])
        qr = tmp.tile([P, 512], BF16, name="qr")
        V.tensor_add(qr[:], t1[:], t2[:])
        ib_ps = psum.tile([P, 512], F32, name="ib_ps", tag="bcast")
        mm1(ib_ps[:], sel2T[:], iq_bf[:])
        V.tensor_mul(qn[:, gsl], qr[:], ib_ps[:])

    def process_k(k_ps, blk, nt):
        gsl = slice(blk * TOK + nt * 512, blk * TOK + (nt + 1) * 512)
        cpos = (blk % 2) * TOK + nt * 512
        csl = slice(cpos, cpos + 512)
        b_idx = blk // 2
        kb = tmp.tile([P, 512], BF16, name="kb")
        S.activation(kb[:], k_ps[:], AF.Copy)
        k2 = tmp.tile([P, 512], BF16, name="k2")
        S.activation(k2[:], kb[:], AF.Square)
        ks_ps = psum.tile([P, 8], F32, name="ks_ps", tag="red")
        for hh in range(2):
            for t4 in range(4):
                mm1(ks_ps[:, hh * 4 + t4:hh * 4 + t4 + 1],
                    k2[hh * 64:(hh + 1) * 64, t4 * 128:(t4 + 1) * 128],
                    ones128[hh * 64:(hh + 1) * 64, 0:1])
        kt0 = (blk % 2) * 8 + nt * 4
        base = (b_idx * 16 + kt0) * 2
        V.tensor_copy(
            invk_raw[:, base:base + 8]
            .rearrange("p (t4 h) -> p h t4", h=2),
            ks_ps[:].rearrange("p (h t4) -> p h t4", h=2))
        kswp_ps = psum.tile([P, 512], F32, name="kswp_ps", tag="bcast")
        mm1(kswp_ps[:], perm_sb[:], kb[:])
        t1 = tmp.tile([P, 512], BF16, name="rope_t1")
        t2 = tmp.tile([P, 512], BF16, name="rope_t2")
        V.tensor_mul(t1[:], kb[:], Cb[:, csl])
        V.tensor_mul(t2[:], kswp_ps[:], Sb[:, csl])
        V.tensor_add(kn[:, gsl], t1[:], t2[:])

    for blk in range(8):
        b_idx = blk // 2
        rhs = {}
        for nt in range(2):
            for ci in range(8):
                r = rhsp.tile([P, 512], BF16, name="h1r")
                dma(out=r[:], in_=h1_all[0][blk, ci * P:(ci + 1) * P,
                                            nt * 512:(nt + 1) * 512])
                rhs[(nt, ci)] = r

        # q/k accumulation; lhsT reused across nt (one LDW per 2 matmuls)
        acc = {}
        for wname in ("q", "k"):
            for nt in range(2):
                acc[(wname, nt)] = psum.tile(
                    [P, 512], F32, name=f"{wname}{nt}_ps",
                    tag=f"{wname}{nt}_ps")
        for ci in range(8):
            for w_off, wname in ((0, "q"), (128, "k")):
                wt = w_sb[:, ci * 384 + w_off:ci * 384 + w_off + 128]
                for nt in range(2):
                    mm(acc[(wname, nt)][:], wt, rhs[(nt, ci)][:],
                       start=(ci == 0), stop=(ci == 7))
        for nt in range(2):
            process_q(acc[("q", nt)], blk, nt)
            process_k(acc[("k", nt)], blk, nt)

        # v: co-major matmul then PE transpose to token-major
        for nt in range(2):
            v_ps = psum.tile([P, 512], F32, name="v_ps", tag="vtp", bufs=2)
            for ci in range(8):
                mm(v_ps[:], w_sb[:, ci * 384 + 256:ci * 384 + 384],
                   rhs[(nt, ci)][:], start=(ci == 0), stop=(ci == 7))
            vco = tmp.tile([P, 512], BF16, name="vco")
            S.activation(vco[:], v_ps[:], AF.Copy)
            kt0 = (blk % 2) * 8 + nt * 4
            for t4 in range(4):
                tp_ps = psum.tile([P, P], BF16, name="tp_ps", tag="vtp",
                                  bufs=2)
                nc.tensor.transpose(tp_ps[:], vco[:, t4 * 128:(t4 + 1) * 128],
                                    ident_sb[:])
                kt = kt0 + t4
                vbase = (b_idx * 16 + kt) * 2 * 65
                V.tensor_copy(
                    v_sb[:, vbase:vbase + 130]
                    .rearrange("p (h c) -> p h c", h=2)[:, :, 0:64],
                    tp_ps[:].rearrange("p (h c) -> p h c", h=2))
    psum_cm.__exit__(None, None, None)


def attention(nc, tc, dbg, slab, tmp, a2a_in, mm, mm1,
              ones65, vb128, qn, kn, v_sb, invk):
    V = nc.vector
    S = nc.scalar
    dma = nc.sync.dma_start
    psum_cm = tc.tile_pool(name="atnpsum", bufs=1, space="PSUM")
    psum = psum_cm.__enter__()
    attn = slab.tile([P, B * L], BF16, name="attn")

    pending = []

    # eviction: per (b, qh, hh, j) the pv (65, 512) -> attn rows hh*64..
    def evict(b2, q2, items):
        for (hh, j, pvall) in items:
            rec = tmp.tile([65, 512], F32, name="rec")
            V.reciprocal_approx_fast(rec[:], pvall[:])
            rec_bf = tmp.tile([65, 512], BF16, name="rec_bf")
            V.tensor_copy(rec_bf[64:65, :], rec[64:65, :])
            rb_ps = psum.tile([P, 1024], F32, name="rb_ps",
                              tag="s_h0")
            mm(rb_ps[hh * 64:hh * 64 + 64, 0:512], ones65[64:65, :],
               rec_bf[64:65, :], start=True, stop=True)
            tm = tmp.tile([P, 512], BF16, name="tm")
            V.tensor_mul(tm[hh * 64:(hh + 1) * 64, :], pvall[0:64, :],
                         rb_ps[hh * 64:hh * 64 + 64, 0:512])
            col = b2 * L + q2 * 1024 + j * 512
            V.tensor_scalar_add(attn[hh * 64:(hh + 1) * 64, col:col + 512],
                                tm[hh * 64:(hh + 1) * 64, :],
                                vb128[hh * 64:(hh + 1) * 64, 0:1])

    for b_idx in range(B):
        for qh in range(2):
            pv = {}
            for hh in range(2):
                for j in range(2):
                    pv[(hh, j)] = psum.tile(
                        [65, 512], F32, name=f"pv{hh}{j}", tag=f"pv{hh}{j}")

            def drain(item):
                ktd, es = item
                for hh in range(2):
                    vb = ((b_idx * 16 + ktd) * 2 + hh) * 65
                    for j in range(2):
                        mm(pv[(hh, j)][:], v_sb[:, vb:vb + 65],
                           es[hh][:, j * 512:(j + 1) * 512],
                           start=(ktd == 0), stop=(ktd == 15))

            pend = []
            for kt in range(16):
                ksl = slice(b_idx * L + kt * 128, b_idx * L + (kt + 1) * 128)
                sh = []
                for hh in range(2):
                    s_h = psum.tile([P, 1024], F32, name=f"s_h{hh}",
                                    tag=f"s_h{hh}")
                    sh.append(s_h)
                # interleave heads so row-groups 0-63 / 64-127 overlap in PE
                for j in range(2):
                    qsl = slice(b_idx * L + qh * 1024 + j * 512,
                                b_idx * L + qh * 1024 + (j + 1) * 512)
                    for hh in range(2):
                        hs = slice(hh * 64, (hh + 1) * 64)
                        mm1(sh[hh][:, j * 512:(j + 1) * 512],
                            kn[hs, ksl], qn[hs, qsl])
                es = []
                for hh in range(2):
                    e_bf = tmp.tile([P, 1024], BF16, name="e_bf", bufs=6)
                    ikcol = (b_idx * 16 + kt) * 2 + hh
                    S.activation(e_bf[:], sh[hh][:], AF.Exp,
                                 scale=invk[:, ikcol:ikcol + 1])
                    es.append(e_bf)
                pend.append((kt, es))
                if len(pend) > 2:
                    drain(pend.pop(0))
                if kt in (6, 8, 10, 12) and pending:
                    b2, q2, items = pending[0]
                    evict(b2, q2, [items.pop(0)])
                    if not items:
                        pending.pop(0)
            for item in pend:
                drain(item)

            items = []
            for hh in range(2):
                for j in range(2):
                    pvall = tmp.tile([65, 512], F32, name="pvall", bufs=8)
                    V.tensor_copy(pvall[:], pv[(hh, j)][:])
                    items.append((hh, j, pvall))
            pending.append((b_idx, qh, items))
    while pending:
        b2, q2, items = pending.pop(0)
        evict(b2, q2, items)

    psum_cm.__exit__(None, None, None)
    if "attn" in dbg:
        dma(out=dbg["attn"][:], in_=attn[:])
    dma(out=a2a_in[:].rearrange("blk p t -> p blk t"),
        in_=attn[:].rearrange("p (blk t) -> p blk t", blk=8))


# ---------------------------------------------------------------------------
# host-side input preparation
# ---------------------------------------------------------------------------

_PERM = np.concatenate([np.arange(0, HD, 2), np.arange(1, HD, 2)])  # re|im


def _perm_matrix():
    """(128,128) with entry (swap(m), m) = 1; swap exchanges the re (0:32)
    and im (32:64) halves of each 64-row head slice."""
    pm = np.zeros((P, P), np.float32)
    for m in range(P):
        base = (m // 64) * 64
        r = m - base
        sw = base + (r + 32) % 64
        pm[sw, m] = 1.0
    return pm


def prep_in_maps(inputs):
    import ml_dtypes
    bf = lambda a: np.ascontiguousarray(a).astype(ml_dtypes.bfloat16)
    f32 = lambda a: np.ascontiguousarray(np.asarray(a, dtype=np.float32))

    x = np.asarray(inputs["x"], np.float32)
    cond = np.asarray(inputs["cond_BD"], np.float32)
    W_qkv = np.asarray(inputs["W_qkv"], np.float32)
    q_bias = np.asarray(inputs["q_bias"], np.float32)
    v_bias = np.asarray(inputs["v_bias"], np.float32)
    sml = np.asarray(inputs["scale_mul_log"], np.float32).reshape(H)
    W_proj = np.asarray(inputs["W_proj"], np.float32)
    b_proj = np.asarray(inputs["b_proj"], np.float32)
    W_fc1 = np.asarray(inputs["W_fc1"], np.float32)
    b_fc1 = np.asarray(inputs["b_fc1"], np.float32)
    W_fc2 = np.asarray(inputs["W_fc2"], np.float32)
    b_fc2 = np.asarray(inputs["b_fc2"], np.float32)
    W_ada = np.asarray(inputs["W_ada"], np.float32)
    b_ada = np.asarray(inputs["b_ada"], np.float32)
    fc = np.asarray(inputs["freqs_cos"], np.float32)
    fs = np.asarray(inputs["freqs_sin"], np.float32)

    cosT = f32(fc.T)   # (32, L)
    sinT = f32(fs.T)
    w_projT = bf(W_proj.T)
    f8 = lambda a: np.ascontiguousarray(a).astype(ml_dtypes.float8_e4m3)
    w_fc1T = f8(W_fc1.T * 64.0)
    w_fc2T = f8(W_fc2.T * 64.0)
    b_proj_c = f32(b_proj.reshape(8, P).T)
    b_fc1_c = f32(b_fc1.reshape(32, P).T)
    b_fc2_c = f32(b_fc2.reshape(8, P).T * 64.0)
    w_adaT = W_ada.T  # (1024, 6144)

    in_maps = []
    for m in range(N_CORES):
        b_own, pm = m // 2, m % 2
        h0, h1 = 2 * m, 2 * m + 1
        cols = []
        for h in (h0, h1):
            cols.append(W_qkv[h * HD + _PERM, :])          # q rows, permuted
        for h in (h0, h1):
            cols.append(W_qkv[C + h * HD + _PERM, :])      # k rows, permuted
        for h in (h0, h1):
            cols.append(W_qkv[2 * C + h * HD:2 * C + (h + 1) * HD, :])  # v
        w_qkv_m = bf(np.concatenate(cols, axis=0).T)       # (1024, 384)
        qkv_b_m = np.concatenate([
            q_bias[h0 * HD + _PERM], q_bias[h1 * HD + _PERM],
            np.zeros(P, np.float32),
            v_bias[h0 * HD:(h0 + 1) * HD], v_bias[h1 * HD:(h1 + 1) * HD],
        ]).reshape(384, 1)

        vb2 = np.stack([v_bias[h0 * HD:(h0 + 1) * HD],
                        v_bias[h1 * HD:(h1 + 1) * HD]], axis=1)
        vb128 = np.concatenate([v_bias[h0 * HD:(h0 + 1) * HD],
                                v_bias[h1 * HD:(h1 + 1) * HD]]).reshape(P, 1)
        xm = x[b_own, pm * TOK:(pm + 1) * TOK, :]          # (1024, 1024)
        identm = np.eye(P, dtype=np.float32)
        s2t = np.zeros((2, P), np.float32)
        s2t[0, 0:64] = 1.0
        s2t[1, 64:128] = 1.0
        in_maps.append({
            "perm_m": bf(_perm_matrix()),
            "sel2t": bf(s2t),
            "ident": bf(identm),
            "vb2": f32(vb2),
            "vb128": f32(vb128),
            "xT": f32(xm.T),
            "condT": f32(cond[b_own].reshape(C, 1)),
            "w_ada": bf(w_adaT[:, pm * 3 * C:(pm + 1) * 3 * C]),
            "b_ada_r": f32(b_ada[pm * 3 * C:(pm + 1) * 3 * C].reshape(1, -1)),
            "w_qkv": w_qkv_m,
            "qkv_b": f32(qkv_b_m),
            "scale_log": f32(sml[[h0, h1]].reshape(2, 1)),
            "cosT": cosT, "sinT": sinT,
            "w_proj": w_projT, "b_proj_c": b_proj_c,
            "w_fc1": w_fc1T, "b_fc1_c": b_fc1_c,
            "w_fc2": w_fc2T, "b_fc2_c": b_fc2_c,
        })
    return in_maps


_NC_CACHE = {}


def _get_nc(debug_outputs=()):
    key = tuple(sorted(debug_outputs))
    if key not in _NC_CACHE:
        _NC_CACHE[key] = build_nc(debug_outputs)
    return _NC_CACHE[key]


def run(inputs, debug_outputs=(), trace=False):
    nc = _get_nc(debug_outputs)
    in_maps = prep_in_maps(inputs)
    res = run_bass_kernel_spmd(nc, in_maps, core_ids=list(range(N_CORES)),
                               trace=trace)
    out = np.empty((B, L, C), np.float32)
    for m in range(N_CORES):
        b_own, pm = m // 2, m % 2
        out[b_own, pm * TOK:(pm + 1) * TOK, :] = res.results[m]["outT"].T
    return out, res


def kernel(**inputs):
    out, _ = run(inputs)
    return out
